# revision 11
# baseline (speedup 1.0000x reference)
"""Bass/Trainium2 kernel for 2-layer GAT (nn_GATa_45260365365735).

Three-launch payload-gather design (8 NeuronCores, SPMD):

  Launch A (node payloads): nodes are range-sharded across cores; each core
    computes PN = x @ wsc for its 12.5k nodes, where wsc [128, 12] packs the
    layer-1 linear algebra collapsed onto the attention vectors:
      cols 0:4  = e_src head logits   (W1 contracted with att_src1)
      cols 4:8  = z     head values   (W1 contracted with W2 — by linearity
                                       layer 2 only consumes h1 @ W2)
      cols 8:12 = e_dst head logits   (W1 contracted with att_dst1)
    Per-edge work therefore needs just 12 values per endpoint instead of the
    128-float feature row, cutting slot DMA ~10x vs gathering x[src].

  Host gathers PN into slot order (pure indexing / replication, as the
  baseline did with x[src]): the slot grid is TRANSPOSED — edge chunks on
  partitions, the 128 lanes (nodes) of a block on the free dim — packed
  densely into S stacks of 128 chunk-rows (blocks may straddle stacks).

  Launch B (layer 1): w = exp(leaky(e_src + e_dst)) per slot-head computed as
    max(exp(u), exp(0.2u)) (exp is monotone, so this IS exp(leaky(u)));
    wz = w*z; the per-destination segment sums become MATMULs with 0/1
    block-indicator stationary matrices (contraction over the chunk partition
    dim), accumulated across stacks into PSUM [NB, 512].  Epilogue:
    h2[d] = sum_h num/den + b1@W2 -> [NB, 128].

  Launch C (layer 2): host gathers h2[src]/h2[dst] into the same slot grid
    (scalar payloads); identical masked-softmax-reduce with heads=1.

  Padded slots ship e_src = -1e4 so exp() underflows to exactly 0 and they
  drop out of numerator and denominator; fully-padded lanes divide 0/0 and
  are discarded by the host inverse-permutation.  The reference's +1e-16 on
  the denominator is skipped: every real lane's denominator >= exp(leaky(
  self-loop logit)) >> 1e-16, so the epsilon is numerically invisible.
"""

import os
import numpy as np
import ml_dtypes

P = 128
N_CORES = 8
HEADS = 4
HID = 32
IN_DIM = 128
NEG_SLOPE = 0.2
KILL = -1.0e4
NMM = 512          # matmul moving free dim (psum bank f32 capacity)

_COMPILED = {}
LAST_EXEC_NS = None
LAST_RESULTS = None


# --------------------------------------------------------------------------
# host preprocessing (indexing / layout / param folding only)
# --------------------------------------------------------------------------

def _structure(edge_index, N):
    """Everything derivable from the graph structure alone."""
    ei = np.asarray(edge_index).astype(np.int64)
    src = np.concatenate([ei[0], np.arange(N, dtype=np.int64)])
    dst = np.concatenate([ei[1], np.arange(N, dtype=np.int64)])
    ET = src.shape[0]

    deg = np.bincount(dst, minlength=N).astype(np.int64)        # >= 1
    order = np.argsort(dst, kind="stable")
    src_sorted = src[order].astype(np.int32)
    estart = np.concatenate([[0], np.cumsum(deg)]).astype(np.int64)

    # round-robin by degree rank -> near-identical degree profiles per core
    grank = np.argsort(-deg, kind="stable")
    per = (N + N_CORES - 1) // N_CORES
    LP = int(np.ceil(per / P) * P)
    NB = LP // P
    perms = []
    for c in range(N_CORES):
        p = grank[c::N_CORES]
        perms.append(np.concatenate(
            [p, np.full(LP - len(p), -1, dtype=np.int64)]))

    CB = np.zeros(NB, dtype=np.int64)
    for c in range(N_CORES):
        d = np.where(perms[c] >= 0, deg[np.maximum(perms[c], 0)], 0)
        CB = np.maximum(CB, d.reshape(NB, P).max(axis=1))
    CB = np.maximum(CB, 1)
    offs = np.concatenate([[0], np.cumsum(CB)]).astype(np.int64)
    T1 = int(offs[-1])
    S = (T1 + P - 1) // P
    R = S * P

    # dense row packing: global chunk-row r belongs to block rowblk[r]
    rowblk = np.full(R, -1, dtype=np.int64)
    for b in range(NB):
        rowblk[offs[b]:offs[b + 1]] = b
    rowchunk = np.arange(R, dtype=np.int64) - np.where(
        rowblk >= 0, offs[np.maximum(rowblk, 0)], 0)

    # per-core slot grids [R, P]: src node id per slot (N = pad)
    lanes = np.arange(P, dtype=np.int64)[None, :]
    srcgrids, dstgrids = [], []
    for c in range(N_CORES):
        pids = perms[c]
        nd = np.where(rowblk[:, None] >= 0,
                      pids[np.maximum(rowblk[:, None], 0) * P + lanes], -1)
        ch = rowchunk[:, None]
        valid = (nd >= 0) & (ch < deg[np.maximum(nd, 0)]) & (rowblk[:, None] >= 0)
        eidx = np.clip(estart[np.maximum(nd, 0)] + ch, 0, ET - 1)
        sg = np.where(valid, src_sorted[eidx].astype(np.int64), N)
        dg = np.where(nd >= 0, nd, N)
        srcgrids.append(sg.astype(np.int32))
        dstgrids.append(dg.astype(np.int32))

    # per-stack block indicator [P, S*NB] (lhsT for segment-sum matmuls)
    indt = np.zeros((P, S * NB), dtype=np.float32)
    for r in range(R):
        b = rowblk[r]
        if b >= 0:
            indt[r % P, (r // P) * NB + b] = 1.0
    indt = indt.astype(ml_dtypes.bfloat16)

    return dict(N=N, LP=LP, NB=NB, T1=T1, S=S, R=R,
                perms=perms, srcgrids=srcgrids, dstgrids=dstgrids,
                indt=indt)


def _fold_params(W1, att_src1, att_dst1, b1, W2, att_src2, att_dst2, b2):
    W1 = np.asarray(W1, dtype=np.float32)
    a_s1 = np.asarray(att_src1, dtype=np.float32)
    a_d1 = np.asarray(att_dst1, dtype=np.float32)
    W2v = np.asarray(W2, dtype=np.float32).reshape(-1)
    W1r = W1.reshape(IN_DIM, HEADS, HID)
    W1a = np.einsum("khc,hc->kh", W1r, a_s1)
    W12 = np.einsum("khf,hf->kh", W1r, W2v.reshape(HEADS, HID))
    W1b = np.einsum("khc,hc->kh", W1r, a_d1)
    wsc = np.concatenate([W1a, W12, W1b], axis=1).astype(ml_dtypes.bfloat16)
    c0 = float(np.asarray(b1, dtype=np.float32).reshape(-1) @ W2v)
    as2 = float(np.asarray(att_src2).reshape(-1)[0])
    ad2 = float(np.asarray(att_dst2).reshape(-1)[0])
    b2f = float(np.asarray(b2).reshape(-1)[0])
    if abs(as2) < 1e-12:
        as2 = 1e-12 if as2 >= 0 else -1e-12
    return wsc, c0, as2, ad2, b2f


# --------------------------------------------------------------------------
# launch A: PN = x @ wsc for a contiguous node range
# --------------------------------------------------------------------------

def _build_a(NAP):
    from contextlib import ExitStack
    import concourse.tile as tile
    from concourse import bacc, mybir

    f32, bf16 = mybir.dt.float32, mybir.dt.bfloat16
    nc = bacc.Bacc("TRN2", target_bir_lowering=False, debug=False,
                   enable_asserts=False, num_devices=N_CORES)
    t_xt = nc.dram_tensor("xt", [P, NAP], bf16, kind="ExternalInput")
    t_wsc = nc.dram_tensor("wsc", [P, 12], bf16, kind="ExternalInput")
    t_pn = nc.dram_tensor("pn", [12, NAP], bf16, kind="ExternalOutput")

    n_mm = NAP // NMM
    CHK = 5 * NMM                   # x chunk: 5 matmuls' worth, in-order DMA
    with tile.TileContext(nc) as tc, ExitStack() as ctx:
        sb = ctx.enter_context(tc.tile_pool(name="sb", bufs=1))
        ps = ctx.enter_context(tc.tile_pool(name="ps", bufs=4, space="PSUM"))
        wsct = sb.tile([P, 12], bf16)
        nc.scalar.dma_start(wsct[:], t_wsc.ap())
        xt = sb.tile([P, NAP], bf16)
        # first chunk small so the matmul stream starts early; in-order on sync
        bounds = [0, 2 * NMM]
        while bounds[-1] < NAP:
            bounds.append(min(bounds[-1] + CHK, NAP))
        for o, e in zip(bounds[:-1], bounds[1:]):
            nc.sync.dma_start(xt[:, o:e], t_xt.ap()[:, o:e])
        po = sb.tile([12, NAP], bf16)
        OC = 5                      # out-DMA every 5 copied chunks
        for k in range(n_mm):
            pk = ps.tile([12, NMM], f32, tag="pk", name=f"pk{k}")
            nc.tensor.matmul(pk[:], lhsT=wsct[:],
                             rhs=xt[:, k * NMM:(k + 1) * NMM],
                             start=True, stop=True)
            dst = po[:, k * NMM:(k + 1) * NMM]
            if k % 2 == 0:
                nc.vector.tensor_copy(dst, pk[:])
            else:
                nc.scalar.copy(dst, pk[:])
            if (k + 1) % OC == 0 or k == n_mm - 1:
                o0 = (k + 1 - ((k % OC) + 1)) * NMM
                o1 = (k + 1) * NMM
                nc.sync.dma_start(t_pn.ap()[:, o0:o1], po[:, o0:o1])
    nc.compile()
    return nc


# --------------------------------------------------------------------------
# launch B: slot payloads -> per-node h2  (layer 1 + W2 collapse)
# --------------------------------------------------------------------------

def _build_b(S, NB, c0):
    from contextlib import ExitStack
    import concourse.tile as tile
    from concourse import bacc, mybir

    f32, bf16 = mybir.dt.float32, mybir.dt.bfloat16
    W = 4 * P                       # 4 head planes of 128 lanes
    nc = bacc.Bacc("TRN2", target_bir_lowering=False, debug=False,
                   enable_asserts=False, num_devices=N_CORES)
    t_g = nc.dram_tensor("gall", [P, S * 2 * W], bf16, kind="ExternalInput")
    t_ind = nc.dram_tensor("indt", [P, S * NB], bf16, kind="ExternalInput")
    t_ind2 = nc.dram_tensor("ind2t", [NB, S * P], bf16, kind="ExternalInput")
    t_ed = nc.dram_tensor("edown", [NB, W], bf16, kind="ExternalInput")
    t_h2 = nc.dram_tensor("h2", [NB, P], f32, kind="ExternalOutput")

    with tile.TileContext(nc) as tc, ExitStack() as ctx:
        sb = ctx.enter_context(tc.tile_pool(name="sb", bufs=1))
        sp = ctx.enter_context(tc.tile_pool(name="sp", bufs=3))
        ps = ctx.enter_context(tc.tile_pool(name="ps", bufs=2, space="PSUM"))
        pe = ctx.enter_context(tc.tile_pool(name="pe", bufs=2, space="PSUM"))

        edown = sb.tile([NB, W], bf16)
        nc.scalar.dma_start(edown[:], t_ed.ap())
        ind2t = sb.tile([NB, S * P], bf16)
        nc.scalar.dma_start(ind2t[:], t_ind2.ap())
        indt = sb.tile([P, S * NB], bf16)
        nc.scalar.dma_start(indt[:], t_ind.ap())
        g = sb.tile([P, S * 2 * W], bf16)
        for s in range(S):          # in-order on one engine: stack s lands s-th
            sl = slice(s * 2 * W, (s + 1) * 2 * W)
            nc.sync.dma_start(g[:, sl], t_g.ap()[:, sl])

        num = ps.tile([NB, W], f32, tag="num")
        den = ps.tile([NB, W], f32, tag="den")

        for s in range(S):
            base = s * 2 * W
            ge = g[:, base:base + W]
            gz = g[:, base + W:base + 2 * W]
            # e_dst replicated down each block's chunk rows by the PE:
            # edrb[row, lane-head] = sum_b ind2t[b, row] * edown[b, lane-head]
            edrb = pe.tile([P, W], f32, tag="edrb", name=f"edrb{s}")
            nc.tensor.matmul(edrb[:], lhsT=ind2t[:, s * P:(s + 1) * P],
                             rhs=edown[:], start=True, stop=True)
            u = sp.tile([P, W], bf16, tag="u", name=f"u{s}")
            nc.vector.tensor_tensor(out=u[:], in0=ge, in1=edrb[:],
                                    op=mybir.AluOpType.add)
            lr = sp.tile([P, W], bf16, tag="lr", name=f"lr{s}")
            nc.vector.scalar_tensor_tensor(
                out=lr[:], in0=u[:], scalar=NEG_SLOPE, in1=u[:],
                op0=mybir.AluOpType.mult, op1=mybir.AluOpType.max)
            w = sp.tile([P, W], bf16, tag="w", name=f"w{s}")
            nc.scalar.activation(w[:], lr[:],
                                 mybir.ActivationFunctionType.Exp)
            lhs = indt[:, s * NB:(s + 1) * NB]
            nc.tensor.matmul(den[:], lhsT=lhs, rhs=w[:],
                             start=(s == 0), stop=(s == S - 1))
            wz = sp.tile([P, W], bf16, tag="wz", name=f"wz{s}")
            nc.vector.tensor_tensor(out=wz[:], in0=w[:], in1=gz,
                                    op=mybir.AluOpType.mult)
            nc.tensor.matmul(num[:], lhsT=lhs, rhs=wz[:],
                             start=(s == 0), stop=(s == S - 1))

        rc = sb.tile([NB, W], f32)
        nc.vector.reciprocal_approx_fast(out=rc[:], in_=den[:])
        nr = sb.tile([NB, W], f32)
        nc.vector.tensor_tensor(out=nr[:], in0=num[:], in1=rc[:],
                                op=mybir.AluOpType.mult)
        h2 = sb.tile([NB, P], f32)
        nc.vector.reduce_sum(
            h2[:], nr[:].rearrange("q (h l) -> q l h", h=HEADS, l=P),
            axis=mybir.AxisListType.X)
        if c0 != 0.0:
            h2o = sb.tile([NB, P], f32)
            nc.vector.tensor_scalar(h2o[:], h2[:], c0, None,
                                    op0=mybir.AluOpType.add)
            h2 = h2o
        nc.sync.dma_start(t_h2.ap()[:], h2[:])
    nc.compile()
    return nc


# --------------------------------------------------------------------------
# launch C: h2 slot payloads -> output  (layer 2, heads=1)
# --------------------------------------------------------------------------

def _build_c(S, NB, as2, ad2, b2f):
    from contextlib import ExitStack
    import concourse.tile as tile
    from concourse import bacc, mybir

    f32, bf16 = mybir.dt.float32, mybir.dt.bfloat16
    nc = bacc.Bacc("TRN2", target_bir_lowering=False, debug=False,
                   enable_asserts=False, num_devices=N_CORES)
    t_g = nc.dram_tensor("g2all", [P, S * 2 * P], bf16, kind="ExternalInput")
    t_ind = nc.dram_tensor("indt", [P, S * NB], bf16, kind="ExternalInput")
    t_out = nc.dram_tensor("out", [NB, P], f32, kind="ExternalOutput")

    ratio = ad2 / as2
    # split stacks into 2 pipeline chunks
    half = (S + 1) // 2
    chunks = [(0, half), (half, S)] if S > 1 else [(0, S)]
    with tile.TileContext(nc) as tc, ExitStack() as ctx:
        sb = ctx.enter_context(tc.tile_pool(name="sb", bufs=1))
        ps = ctx.enter_context(tc.tile_pool(name="ps", bufs=2, space="PSUM"))

        indt = sb.tile([P, S * NB], bf16)
        nc.scalar.dma_start(indt[:], t_ind.ap())
        g = sb.tile([P, S * 2 * P], bf16)
        for (s0, s1) in chunks:
            nc.sync.dma_start(g[:, s0 * 2 * P:s1 * 2 * P],
                              t_g.ap()[:, s0 * 2 * P:s1 * 2 * P])

        num = ps.tile([NB, P], f32, tag="num")
        den = ps.tile([NB, P], f32, tag="den")
        w = sb.tile([P, S * P], bf16)
        wg = sb.tile([P, S * P], bf16)

        first = True
        for (s0, s1) in chunks:
            n = s1 - s0
            gv = g[:, s0 * 2 * P:s1 * 2 * P].rearrange(
                "p (s t l) -> p s t l", s=n, t=2, l=P)
            g2 = gv[:, :, 0, :]
            hr = gv[:, :, 1, :]
            v = sb.tile([P, n * P], f32, name=f"v{s0}")
            v3 = v[:].rearrange("p (s l) -> p s l", s=n, l=P)
            nc.vector.scalar_tensor_tensor(
                out=v3, in0=hr, scalar=ratio, in1=g2,
                op0=mybir.AluOpType.mult, op1=mybir.AluOpType.add)
            e1 = sb.tile([P, n * P], bf16, name=f"e1{s0}")
            nc.scalar.activation(e1[:], v[:],
                                 mybir.ActivationFunctionType.Exp, scale=as2)
            e2 = sb.tile([P, n * P], bf16, name=f"e2{s0}")
            nc.scalar.activation(e2[:], v[:],
                                 mybir.ActivationFunctionType.Exp,
                                 scale=as2 * NEG_SLOPE)
            wv = w[:, s0 * P:s1 * P]
            nc.vector.tensor_tensor(out=wv, in0=e1[:], in1=e2[:],
                                    op=mybir.AluOpType.max)
            wgv = wg[:, s0 * P:s1 * P].rearrange("p (s l) -> p s l", s=n, l=P)
            nc.vector.tensor_tensor(
                out=wgv, in0=w[:, s0 * P:s1 * P].rearrange(
                    "p (s l) -> p s l", s=n, l=P),
                in1=g2, op=mybir.AluOpType.mult)
            for s in range(s0, s1):
                lhs = indt[:, s * NB:(s + 1) * NB]
                nc.tensor.matmul(den[:], lhsT=lhs,
                                 rhs=w[:, s * P:(s + 1) * P],
                                 start=(s == 0), stop=(s == S - 1))
                nc.tensor.matmul(num[:], lhsT=lhs,
                                 rhs=wg[:, s * P:(s + 1) * P],
                                 start=(s == 0), stop=(s == S - 1))
            first = False

        rc = sb.tile([NB, P], f32)
        nc.vector.reciprocal_approx_fast(out=rc[:], in_=den[:])
        o = sb.tile([NB, P], f32)
        nc.vector.tensor_tensor(out=o[:], in0=num[:], in1=rc[:],
                                op=mybir.AluOpType.mult)
        if b2f != 0.0:
            ob = sb.tile([NB, P], f32)
            nc.vector.tensor_scalar(ob[:], o[:], b2f, None,
                                    op0=mybir.AluOpType.add)
            o = ob
        nc.sync.dma_start(t_out.ap()[:], o[:])
    nc.compile()
    return nc


# --------------------------------------------------------------------------
# entry point
# --------------------------------------------------------------------------

def _install_ntff_shim():
    """Optional: register the axon NTFF profiling hook (dev tracing only)."""
    import sys as _sys
    import types as _types
    if "antenv.axon_hooks" in _sys.modules:
        return
    try:
        import antenv
        mod = _types.ModuleType("antenv.axon_hooks")
        _state = {"hook": None}
        mod.set_axon_ntff_profile_hook = lambda h: _state.__setitem__("hook", h)
        mod.get_axon_ntff_profile_hook = lambda: _state["hook"]
        _sys.modules["antenv.axon_hooks"] = mod
        antenv.axon_hooks = mod
        from trn_agent_boot.trn_boot import _ntff_profile_via_ctypes
        mod.set_axon_ntff_profile_hook(
            _ntff_profile_via_ctypes("/opt/axon/libaxon_pjrt.so"))
    except Exception as e:  # pragma: no cover
        print("ntff shim unavailable:", e)


def kernel(**inputs):
    global LAST_EXEC_NS, LAST_RESULTS
    from concourse import bass_utils

    x = np.asarray(inputs["x"], dtype=np.float32)
    N = x.shape[0]
    st = _structure(inputs["edge_index"], N)
    wsc, c0, as2, ad2, b2f = _fold_params(
        inputs["W1"], inputs["att_src1"], inputs["att_dst1"], inputs["b1"],
        inputs["W2"], inputs["att_src2"], inputs["att_dst2"], inputs["b2"])

    S, NB, R, LP = st["S"], st["NB"], st["R"], st["LP"]
    per = (N + N_CORES - 1) // N_CORES
    NAP = ((per + NMM - 1) // NMM) * NMM

    key = (N, S, NB, st["T1"], round(c0, 9), round(as2, 12),
           round(ad2, 12), round(b2f, 9))
    if key not in _COMPILED:
        _COMPILED[key] = (_build_a(NAP), _build_b(S, NB, c0),
                          _build_c(S, NB, as2, ad2, b2f))
    nca, ncb, ncc = _COMPILED[key]

    trace = os.environ.get("GAT_TRACE", "0") == "1"
    if trace:
        _install_ntff_shim()

    # ---- launch A
    xbf = x.astype(ml_dtypes.bfloat16)
    in_a = []
    for c in range(N_CORES):
        lo = c * per
        xt = np.zeros((P, NAP), dtype=ml_dtypes.bfloat16)
        n_c = min(per, N - lo)
        xt[:, :n_c] = xbf[lo:lo + n_c].T
        in_a.append({"xt": xt, "wsc": np.asarray(wsc)})
    res_a = bass_utils.run_bass_kernel_spmd(
        nca, in_a, core_ids=list(range(N_CORES)), trace=trace)

    # host: assemble padded payload table [12, N+1] (col N = padding)
    pn_bf = np.zeros((12, N + 1), dtype=ml_dtypes.bfloat16)
    for c in range(N_CORES):
        lo = c * per
        n_c = min(per, N - lo)
        pn_bf[:, lo:lo + n_c] = res_a.results[c]["pn"][:, :n_c]
    pn_bf[0:4, N] = KILL

    # ---- launch B inputs: gather payloads into slot grids
    # per-stack block indicator transpose [NB, S*P] (EDR broadcast lhsT)
    ind2t = np.ascontiguousarray(
        np.asarray(st["indt"], dtype=np.float32)
        .reshape(P, S, NB).transpose(2, 1, 0)
        .reshape(NB, S * P)).astype(ml_dtypes.bfloat16)
    in_b = []
    for c in range(N_CORES):
        sub = pn_bf[:, st["srcgrids"][c]]            # [12, R, 128]
        big = np.stack([sub[0:4], sub[4:8]], axis=0)        # [sec, h, R, l]
        big = big.reshape(2, 4, S, P, P)
        gall = np.ascontiguousarray(
            big.transpose(3, 2, 0, 1, 4)).reshape(P, S * 2 * 4 * P)
        nodes = np.where(st["perms"][c] >= 0, st["perms"][c], N)
        edown = np.ascontiguousarray(
            pn_bf[8:12][:, nodes].reshape(4, NB, P)
            .transpose(1, 0, 2)).reshape(NB, 4 * P)
        in_b.append({"gall": gall, "indt": np.asarray(st["indt"]),
                     "ind2t": ind2t, "edown": edown})
    res_b = bass_utils.run_bass_kernel_spmd(
        ncb, in_b, core_ids=list(range(N_CORES)), trace=trace)

    # host: scatter h2 back to node order, with kill/zero padded tables
    h2_node = np.zeros(N, dtype=np.float32)
    for c in range(N_CORES):
        h2v = res_b.results[c]["h2"].reshape(-1)     # [NB*P] block-major
        real = st["perms"][c] >= 0
        h2_node[st["perms"][c][real]] = h2v[real]
    h2_kill = np.zeros(N + 1, dtype=np.float32)
    h2_kill[:N] = h2_node
    h2_kill[N] = 2.0 * KILL / as2
    h2_zero = np.zeros(N + 1, dtype=np.float32)
    h2_zero[:N] = h2_node
    h2k_bf = h2_kill.astype(ml_dtypes.bfloat16)
    h2z_bf = h2_zero.astype(ml_dtypes.bfloat16)

    # ---- launch C inputs
    in_c = []
    for c in range(N_CORES):
        g2 = h2k_bf[st["srcgrids"][c]].reshape(S, P, P)
        hr = h2z_bf[st["dstgrids"][c]].reshape(S, P, P)
        ga = np.stack([g2, hr], axis=0)              # [t, s, p, l]
        g2all = np.ascontiguousarray(
            ga.transpose(2, 1, 0, 3)).reshape(P, S * 2 * P)
        in_c.append({"g2all": g2all, "indt": np.asarray(st["indt"])})
    res_c = bass_utils.run_bass_kernel_spmd(
        ncc, in_c, core_ids=list(range(N_CORES)), trace=trace)

    out = np.zeros((N, 1), dtype=np.float32)
    for c in range(N_CORES):
        ov = res_c.results[c]["out"].reshape(-1)
        real = st["perms"][c] >= 0
        out[st["perms"][c][real], 0] = ov[real]

    ts = [r.exec_time_ns for r in (res_a, res_b, res_c)]
    LAST_EXEC_NS = sum(t for t in ts if t) if any(ts) else None
    LAST_RESULTS = (res_a, res_b, res_c)
    return out


# revision 13
# speedup vs baseline: 1.0164x; 1.0164x over previous
"""Bass/Trainium2 kernel for 2-layer GAT (nn_GATa_45260365365735).

Three-launch payload-gather design (8 NeuronCores, SPMD):

  Launch A (node payloads): nodes are range-sharded across cores; each core
    computes PN = x @ wsc for its 12.5k nodes, where wsc [128, 12] packs the
    layer-1 linear algebra collapsed onto the attention vectors:
      cols 0:4  = e_src head logits   (W1 contracted with att_src1)
      cols 4:8  = z     head values   (W1 contracted with W2 — by linearity
                                       layer 2 only consumes h1 @ W2)
      cols 8:12 = e_dst head logits   (W1 contracted with att_dst1)
    Per-edge work therefore needs just 12 values per endpoint instead of the
    128-float feature row, cutting slot DMA ~10x vs gathering x[src].

  Host gathers PN into slot order (pure indexing / replication, as the
  baseline did with x[src]): the slot grid is TRANSPOSED — edge chunks on
  partitions, the 128 lanes (nodes) of a block on the free dim — packed
  densely into S stacks of 128 chunk-rows (blocks may straddle stacks).

  Launch B (layer 1): w = exp(leaky(e_src + e_dst)) per slot-head computed as
    max(exp(u), exp(0.2u)) (exp is monotone, so this IS exp(leaky(u)));
    wz = w*z; the per-destination segment sums become MATMULs with 0/1
    block-indicator stationary matrices (contraction over the chunk partition
    dim), accumulated across stacks into PSUM [NB, 512].  Epilogue:
    h2[d] = sum_h num/den + b1@W2 -> [NB, 128].

  Launch C (layer 2): host gathers h2[src]/h2[dst] into the same slot grid
    (scalar payloads); identical masked-softmax-reduce with heads=1.

  Padded slots ship e_src = -1e4 so exp() underflows to exactly 0 and they
  drop out of numerator and denominator; fully-padded lanes divide 0/0 and
  are discarded by the host inverse-permutation.  The reference's +1e-16 on
  the denominator is skipped: every real lane's denominator >= exp(leaky(
  self-loop logit)) >> 1e-16, so the epsilon is numerically invisible.
"""

import os
import numpy as np
import ml_dtypes

P = 128
N_CORES = 8
HEADS = 4
HID = 32
IN_DIM = 128
NEG_SLOPE = 0.2
KILL = -1.0e4
NMM = 512          # matmul moving free dim (psum bank f32 capacity)

_COMPILED = {}
LAST_EXEC_NS = None
LAST_RESULTS = None


# --------------------------------------------------------------------------
# host preprocessing (indexing / layout / param folding only)
# --------------------------------------------------------------------------

def _structure(edge_index, N):
    """Everything derivable from the graph structure alone."""
    ei = np.asarray(edge_index).astype(np.int64)
    src = np.concatenate([ei[0], np.arange(N, dtype=np.int64)])
    dst = np.concatenate([ei[1], np.arange(N, dtype=np.int64)])
    ET = src.shape[0]

    deg = np.bincount(dst, minlength=N).astype(np.int64)        # >= 1
    order = np.argsort(dst, kind="stable")
    src_sorted = src[order].astype(np.int32)
    estart = np.concatenate([[0], np.cumsum(deg)]).astype(np.int64)

    # round-robin by degree rank -> near-identical degree profiles per core
    grank = np.argsort(-deg, kind="stable")
    per = (N + N_CORES - 1) // N_CORES
    LP = int(np.ceil(per / P) * P)
    NB = LP // P
    perms = []
    for c in range(N_CORES):
        p = grank[c::N_CORES]
        perms.append(np.concatenate(
            [p, np.full(LP - len(p), -1, dtype=np.int64)]))

    CB = np.zeros(NB, dtype=np.int64)
    for c in range(N_CORES):
        d = np.where(perms[c] >= 0, deg[np.maximum(perms[c], 0)], 0)
        CB = np.maximum(CB, d.reshape(NB, P).max(axis=1))
    CB = np.maximum(CB, 1)
    offs = np.concatenate([[0], np.cumsum(CB)]).astype(np.int64)
    T1 = int(offs[-1])
    S = (T1 + P - 1) // P
    R = S * P

    # dense row packing: global chunk-row r belongs to block rowblk[r]
    rowblk = np.full(R, -1, dtype=np.int64)
    for b in range(NB):
        rowblk[offs[b]:offs[b + 1]] = b
    rowchunk = np.arange(R, dtype=np.int64) - np.where(
        rowblk >= 0, offs[np.maximum(rowblk, 0)], 0)

    # per-core slot grids [R, P]: src node id per slot (N = pad)
    lanes = np.arange(P, dtype=np.int64)[None, :]
    srcgrids, dstgrids = [], []
    for c in range(N_CORES):
        pids = perms[c]
        nd = np.where(rowblk[:, None] >= 0,
                      pids[np.maximum(rowblk[:, None], 0) * P + lanes], -1)
        ch = rowchunk[:, None]
        valid = (nd >= 0) & (ch < deg[np.maximum(nd, 0)]) & (rowblk[:, None] >= 0)
        eidx = np.clip(estart[np.maximum(nd, 0)] + ch, 0, ET - 1)
        sg = np.where(valid, src_sorted[eidx].astype(np.int64), N)
        dg = np.where(nd >= 0, nd, N)
        srcgrids.append(sg.astype(np.int32))
        dstgrids.append(dg.astype(np.int32))

    # per-stack block indicator [P, S*NB] (lhsT for segment-sum matmuls)
    indt = np.zeros((P, S * NB), dtype=np.float32)
    for r in range(R):
        b = rowblk[r]
        if b >= 0:
            indt[r % P, (r // P) * NB + b] = 1.0
    indt = indt.astype(ml_dtypes.bfloat16)

    return dict(N=N, LP=LP, NB=NB, T1=T1, S=S, R=R,
                perms=perms, srcgrids=srcgrids, dstgrids=dstgrids,
                indt=indt)


def _fold_params(W1, att_src1, att_dst1, b1, W2, att_src2, att_dst2, b2):
    W1 = np.asarray(W1, dtype=np.float32)
    a_s1 = np.asarray(att_src1, dtype=np.float32)
    a_d1 = np.asarray(att_dst1, dtype=np.float32)
    W2v = np.asarray(W2, dtype=np.float32).reshape(-1)
    W1r = W1.reshape(IN_DIM, HEADS, HID)
    W1a = np.einsum("khc,hc->kh", W1r, a_s1)
    W12 = np.einsum("khf,hf->kh", W1r, W2v.reshape(HEADS, HID))
    W1b = np.einsum("khc,hc->kh", W1r, a_d1)
    wsc = np.concatenate([W1a, W12, W1b], axis=1).astype(ml_dtypes.bfloat16)
    c0 = float(np.asarray(b1, dtype=np.float32).reshape(-1) @ W2v)
    as2 = float(np.asarray(att_src2).reshape(-1)[0])
    ad2 = float(np.asarray(att_dst2).reshape(-1)[0])
    b2f = float(np.asarray(b2).reshape(-1)[0])
    if abs(as2) < 1e-12:
        as2 = 1e-12 if as2 >= 0 else -1e-12
    return wsc, c0, as2, ad2, b2f


# --------------------------------------------------------------------------
# launch A: PN = x @ wsc for a contiguous node range
# --------------------------------------------------------------------------

def _build_a(NAP):
    from contextlib import ExitStack
    import concourse.tile as tile
    from concourse import bacc, mybir

    f32, bf16 = mybir.dt.float32, mybir.dt.bfloat16
    nc = bacc.Bacc("TRN2", target_bir_lowering=False, debug=False,
                   enable_asserts=False, num_devices=N_CORES)
    t_xt = nc.dram_tensor("xt", [P, NAP], bf16, kind="ExternalInput")
    t_wsc = nc.dram_tensor("wsc", [P, 12], bf16, kind="ExternalInput")
    t_pn = nc.dram_tensor("pn", [12, NAP], bf16, kind="ExternalOutput")

    n_mm = NAP // NMM
    CHK = 5 * NMM                   # x chunk: 5 matmuls' worth, in-order DMA
    with tile.TileContext(nc) as tc, ExitStack() as ctx:
        sb = ctx.enter_context(tc.tile_pool(name="sb", bufs=1))
        ps = ctx.enter_context(tc.tile_pool(name="ps", bufs=4, space="PSUM"))
        wsct = sb.tile([P, 12], bf16)
        nc.scalar.dma_start(wsct[:], t_wsc.ap())
        xt = sb.tile([P, NAP], bf16)
        # first chunk small so the matmul stream starts early; in-order on sync
        bounds = [0, 2 * NMM]
        while bounds[-1] < NAP:
            bounds.append(min(bounds[-1] + CHK, NAP))
        for o, e in zip(bounds[:-1], bounds[1:]):
            nc.sync.dma_start(xt[:, o:e], t_xt.ap()[:, o:e])
        po = sb.tile([12, NAP], bf16)
        OC = 5                      # out-DMA every 5 copied chunks
        for k in range(n_mm):
            pk = ps.tile([12, NMM], f32, tag="pk", name=f"pk{k}")
            nc.tensor.matmul(pk[:], lhsT=wsct[:],
                             rhs=xt[:, k * NMM:(k + 1) * NMM],
                             start=True, stop=True)
            dst = po[:, k * NMM:(k + 1) * NMM]
            if k % 2 == 0:
                nc.vector.tensor_copy(dst, pk[:])
            else:
                nc.scalar.copy(dst, pk[:])
            if (k + 1) % OC == 0 or k == n_mm - 1:
                o0 = (k + 1 - ((k % OC) + 1)) * NMM
                o1 = (k + 1) * NMM
                nc.sync.dma_start(t_pn.ap()[:, o0:o1], po[:, o0:o1])
    nc.compile()
    return nc


# --------------------------------------------------------------------------
# launch B: slot payloads -> per-node h2  (layer 1 + W2 collapse)
# --------------------------------------------------------------------------

def _build_b(S, NB, c0):
    from contextlib import ExitStack
    import concourse.tile as tile
    from concourse import bacc, mybir

    f32, bf16 = mybir.dt.float32, mybir.dt.bfloat16
    W = 4 * P                       # 4 head planes of 128 lanes
    nc = bacc.Bacc("TRN2", target_bir_lowering=False, debug=False,
                   enable_asserts=False, num_devices=N_CORES)
    t_g = nc.dram_tensor("gall", [P, S * 2 * W], bf16, kind="ExternalInput")
    t_ind = nc.dram_tensor("indt", [P, S * NB], bf16, kind="ExternalInput")
    t_ind2 = nc.dram_tensor("ind2t", [NB, S * P], bf16, kind="ExternalInput")
    t_ed = nc.dram_tensor("edown", [NB, W], bf16, kind="ExternalInput")
    t_h2 = nc.dram_tensor("h2", [NB, P], f32, kind="ExternalOutput")

    with tile.TileContext(nc) as tc, ExitStack() as ctx:
        sb = ctx.enter_context(tc.tile_pool(name="sb", bufs=1))
        sp = ctx.enter_context(tc.tile_pool(name="sp", bufs=3))
        ps = ctx.enter_context(tc.tile_pool(name="ps", bufs=2, space="PSUM"))
        pe = ctx.enter_context(tc.tile_pool(name="pe", bufs=2, space="PSUM"))

        # all loads on one head: queue FIFO makes arrival follow issue order,
        # so the small broadcast/indicator tensors land before the bulk data
        edown = sb.tile([NB, W], bf16)
        nc.sync.dma_start(edown[:], t_ed.ap())
        ind2t = sb.tile([NB, S * P], bf16)
        nc.sync.dma_start(ind2t[:], t_ind2.ap())
        indt = sb.tile([P, S * NB], bf16)
        nc.sync.dma_start(indt[:], t_ind.ap())
        g = sb.tile([P, S * 2 * W], bf16)
        for s in range(S):
            sl = slice(s * 2 * W, (s + 1) * 2 * W)
            nc.sync.dma_start(g[:, sl], t_g.ap()[:, sl])

        num = ps.tile([NB, W], f32, tag="num")
        den = ps.tile([NB, W], f32, tag="den")

        for s in range(S):
            base = s * 2 * W
            ge = g[:, base:base + W]
            gz = g[:, base + W:base + 2 * W]
            # e_dst replicated down each block's chunk rows by the PE:
            # edrb[row, lane-head] = sum_b ind2t[b, row] * edown[b, lane-head]
            edrb = pe.tile([P, W], f32, tag="edrb", name=f"edrb{s}")
            nc.tensor.matmul(edrb[:], lhsT=ind2t[:, s * P:(s + 1) * P],
                             rhs=edown[:], start=True, stop=True)
            u = sp.tile([P, W], bf16, tag="u", name=f"u{s}")
            nc.vector.tensor_tensor(out=u[:], in0=ge, in1=edrb[:],
                                    op=mybir.AluOpType.add)
            # exp(leaky(u)) == max(exp(u), exp(0.2u)) — both exps on scalar
            e1 = sp.tile([P, W], bf16, tag="e1", name=f"e1{s}")
            nc.scalar.activation(e1[:], u[:],
                                 mybir.ActivationFunctionType.Exp)
            e2 = sp.tile([P, W], bf16, tag="e2", name=f"e2{s}")
            nc.scalar.activation(e2[:], u[:],
                                 mybir.ActivationFunctionType.Exp,
                                 scale=NEG_SLOPE)
            w = sp.tile([P, W], bf16, tag="w", name=f"w{s}")
            nc.vector.tensor_tensor(out=w[:], in0=e1[:], in1=e2[:],
                                    op=mybir.AluOpType.max)
            lhs = indt[:, s * NB:(s + 1) * NB]
            nc.tensor.matmul(den[:], lhsT=lhs, rhs=w[:],
                             start=(s == 0), stop=(s == S - 1))
            wz = sp.tile([P, W], bf16, tag="wz", name=f"wz{s}")
            nc.vector.tensor_tensor(out=wz[:], in0=w[:], in1=gz,
                                    op=mybir.AluOpType.mult)
            nc.tensor.matmul(num[:], lhsT=lhs, rhs=wz[:],
                             start=(s == 0), stop=(s == S - 1))

        rc = sb.tile([NB, W], f32)
        nc.vector.reciprocal_approx_fast(out=rc[:], in_=den[:])
        nr = sb.tile([NB, W], f32)
        nc.vector.tensor_tensor(out=nr[:], in0=num[:], in1=rc[:],
                                op=mybir.AluOpType.mult)
        h2 = sb.tile([NB, P], f32)
        nc.vector.reduce_sum(
            h2[:], nr[:].rearrange("q (h l) -> q l h", h=HEADS, l=P),
            axis=mybir.AxisListType.X)
        if c0 != 0.0:
            h2o = sb.tile([NB, P], f32)
            nc.vector.tensor_scalar(h2o[:], h2[:], c0, None,
                                    op0=mybir.AluOpType.add)
            h2 = h2o
        nc.sync.dma_start(t_h2.ap()[:], h2[:])
    nc.compile()
    return nc


# --------------------------------------------------------------------------
# launch C: h2 slot payloads -> output  (layer 2, heads=1)
# --------------------------------------------------------------------------

def _build_c(S, NB, as2, ad2, b2f):
    from contextlib import ExitStack
    import concourse.tile as tile
    from concourse import bacc, mybir

    f32, bf16 = mybir.dt.float32, mybir.dt.bfloat16
    nc = bacc.Bacc("TRN2", target_bir_lowering=False, debug=False,
                   enable_asserts=False, num_devices=N_CORES)
    t_g = nc.dram_tensor("g2all", [P, S * 2 * P], bf16, kind="ExternalInput")
    t_ind = nc.dram_tensor("indt", [P, S * NB], bf16, kind="ExternalInput")
    t_out = nc.dram_tensor("out", [NB, P], f32, kind="ExternalOutput")

    ratio = ad2 / as2
    # split stacks into 2 pipeline chunks
    half = (S + 1) // 2
    chunks = [(0, half), (half, S)] if S > 1 else [(0, S)]
    with tile.TileContext(nc) as tc, ExitStack() as ctx:
        sb = ctx.enter_context(tc.tile_pool(name="sb", bufs=1))
        ps = ctx.enter_context(tc.tile_pool(name="ps", bufs=2, space="PSUM"))

        indt = sb.tile([P, S * NB], bf16)
        nc.scalar.dma_start(indt[:], t_ind.ap())
        g = sb.tile([P, S * 2 * P], bf16)
        for (s0, s1) in chunks:
            nc.sync.dma_start(g[:, s0 * 2 * P:s1 * 2 * P],
                              t_g.ap()[:, s0 * 2 * P:s1 * 2 * P])

        num = ps.tile([NB, P], f32, tag="num")
        den = ps.tile([NB, P], f32, tag="den")
        w = sb.tile([P, S * P], bf16)
        wg = sb.tile([P, S * P], bf16)

        first = True
        for (s0, s1) in chunks:
            n = s1 - s0
            gv = g[:, s0 * 2 * P:s1 * 2 * P].rearrange(
                "p (s t l) -> p s t l", s=n, t=2, l=P)
            g2 = gv[:, :, 0, :]
            hr = gv[:, :, 1, :]
            v = sb.tile([P, n * P], f32, name=f"v{s0}")
            v3 = v[:].rearrange("p (s l) -> p s l", s=n, l=P)
            nc.vector.scalar_tensor_tensor(
                out=v3, in0=hr, scalar=ratio, in1=g2,
                op0=mybir.AluOpType.mult, op1=mybir.AluOpType.add)
            e1 = sb.tile([P, n * P], bf16, name=f"e1{s0}")
            nc.scalar.activation(e1[:], v[:],
                                 mybir.ActivationFunctionType.Exp, scale=as2)
            e2 = sb.tile([P, n * P], bf16, name=f"e2{s0}")
            nc.scalar.activation(e2[:], v[:],
                                 mybir.ActivationFunctionType.Exp,
                                 scale=as2 * NEG_SLOPE)
            wv = w[:, s0 * P:s1 * P]
            nc.vector.tensor_tensor(out=wv, in0=e1[:], in1=e2[:],
                                    op=mybir.AluOpType.max)
            wgv = wg[:, s0 * P:s1 * P].rearrange("p (s l) -> p s l", s=n, l=P)
            nc.vector.tensor_tensor(
                out=wgv, in0=w[:, s0 * P:s1 * P].rearrange(
                    "p (s l) -> p s l", s=n, l=P),
                in1=g2, op=mybir.AluOpType.mult)
            for s in range(s0, s1):
                lhs = indt[:, s * NB:(s + 1) * NB]
                nc.tensor.matmul(den[:], lhsT=lhs,
                                 rhs=w[:, s * P:(s + 1) * P],
                                 start=(s == 0), stop=(s == S - 1))
                nc.tensor.matmul(num[:], lhsT=lhs,
                                 rhs=wg[:, s * P:(s + 1) * P],
                                 start=(s == 0), stop=(s == S - 1))
            first = False

        rc = sb.tile([NB, P], f32)
        nc.vector.reciprocal_approx_fast(out=rc[:], in_=den[:])
        o = sb.tile([NB, P], f32)
        nc.vector.tensor_tensor(out=o[:], in0=num[:], in1=rc[:],
                                op=mybir.AluOpType.mult)
        if b2f != 0.0:
            ob = sb.tile([NB, P], f32)
            nc.vector.tensor_scalar(ob[:], o[:], b2f, None,
                                    op0=mybir.AluOpType.add)
            o = ob
        nc.sync.dma_start(t_out.ap()[:], o[:])
    nc.compile()
    return nc


# --------------------------------------------------------------------------
# entry point
# --------------------------------------------------------------------------

def _install_ntff_shim():
    """Optional: register the axon NTFF profiling hook (dev tracing only)."""
    import sys as _sys
    import types as _types
    if "antenv.axon_hooks" in _sys.modules:
        return
    try:
        import antenv
        mod = _types.ModuleType("antenv.axon_hooks")
        _state = {"hook": None}
        mod.set_axon_ntff_profile_hook = lambda h: _state.__setitem__("hook", h)
        mod.get_axon_ntff_profile_hook = lambda: _state["hook"]
        _sys.modules["antenv.axon_hooks"] = mod
        antenv.axon_hooks = mod
        from trn_agent_boot.trn_boot import _ntff_profile_via_ctypes
        mod.set_axon_ntff_profile_hook(
            _ntff_profile_via_ctypes("/opt/axon/libaxon_pjrt.so"))
    except Exception as e:  # pragma: no cover
        print("ntff shim unavailable:", e)


def kernel(**inputs):
    global LAST_EXEC_NS, LAST_RESULTS
    from concourse import bass_utils

    x = np.asarray(inputs["x"], dtype=np.float32)
    N = x.shape[0]
    st = _structure(inputs["edge_index"], N)
    wsc, c0, as2, ad2, b2f = _fold_params(
        inputs["W1"], inputs["att_src1"], inputs["att_dst1"], inputs["b1"],
        inputs["W2"], inputs["att_src2"], inputs["att_dst2"], inputs["b2"])

    S, NB, R, LP = st["S"], st["NB"], st["R"], st["LP"]
    per = (N + N_CORES - 1) // N_CORES
    NAP = ((per + NMM - 1) // NMM) * NMM

    key = (N, S, NB, st["T1"], round(c0, 9), round(as2, 12),
           round(ad2, 12), round(b2f, 9))
    if key not in _COMPILED:
        _COMPILED[key] = (_build_a(NAP), _build_b(S, NB, c0),
                          _build_c(S, NB, as2, ad2, b2f))
    nca, ncb, ncc = _COMPILED[key]

    trace = os.environ.get("GAT_TRACE", "0") == "1"
    if trace:
        _install_ntff_shim()

    # ---- launch A
    xbf = x.astype(ml_dtypes.bfloat16)
    in_a = []
    for c in range(N_CORES):
        lo = c * per
        xt = np.zeros((P, NAP), dtype=ml_dtypes.bfloat16)
        n_c = min(per, N - lo)
        xt[:, :n_c] = xbf[lo:lo + n_c].T
        in_a.append({"xt": xt, "wsc": np.asarray(wsc)})
    res_a = bass_utils.run_bass_kernel_spmd(
        nca, in_a, core_ids=list(range(N_CORES)), trace=trace)

    # host: assemble padded payload table [12, N+1] (col N = padding)
    pn_bf = np.zeros((12, N + 1), dtype=ml_dtypes.bfloat16)
    for c in range(N_CORES):
        lo = c * per
        n_c = min(per, N - lo)
        pn_bf[:, lo:lo + n_c] = res_a.results[c]["pn"][:, :n_c]
    pn_bf[0:4, N] = KILL

    # ---- launch B inputs: gather payloads into slot grids
    # per-stack block indicator transpose [NB, S*P] (EDR broadcast lhsT)
    ind2t = np.ascontiguousarray(
        np.asarray(st["indt"], dtype=np.float32)
        .reshape(P, S, NB).transpose(2, 1, 0)
        .reshape(NB, S * P)).astype(ml_dtypes.bfloat16)
    in_b = []
    for c in range(N_CORES):
        sub = pn_bf[:, st["srcgrids"][c]]            # [12, R, 128]
        big = np.stack([sub[0:4], sub[4:8]], axis=0)        # [sec, h, R, l]
        big = big.reshape(2, 4, S, P, P)
        gall = np.ascontiguousarray(
            big.transpose(3, 2, 0, 1, 4)).reshape(P, S * 2 * 4 * P)
        nodes = np.where(st["perms"][c] >= 0, st["perms"][c], N)
        edown = np.ascontiguousarray(
            pn_bf[8:12][:, nodes].reshape(4, NB, P)
            .transpose(1, 0, 2)).reshape(NB, 4 * P)
        in_b.append({"gall": gall, "indt": np.asarray(st["indt"]),
                     "ind2t": ind2t, "edown": edown})
    res_b = bass_utils.run_bass_kernel_spmd(
        ncb, in_b, core_ids=list(range(N_CORES)), trace=trace)

    # host: scatter h2 back to node order, with kill/zero padded tables
    h2_node = np.zeros(N, dtype=np.float32)
    for c in range(N_CORES):
        h2v = res_b.results[c]["h2"].reshape(-1)     # [NB*P] block-major
        real = st["perms"][c] >= 0
        h2_node[st["perms"][c][real]] = h2v[real]
    h2_kill = np.zeros(N + 1, dtype=np.float32)
    h2_kill[:N] = h2_node
    h2_kill[N] = 2.0 * KILL / as2
    h2_zero = np.zeros(N + 1, dtype=np.float32)
    h2_zero[:N] = h2_node
    h2k_bf = h2_kill.astype(ml_dtypes.bfloat16)
    h2z_bf = h2_zero.astype(ml_dtypes.bfloat16)

    # ---- launch C inputs
    in_c = []
    for c in range(N_CORES):
        g2 = h2k_bf[st["srcgrids"][c]].reshape(S, P, P)
        hr = h2z_bf[st["dstgrids"][c]].reshape(S, P, P)
        ga = np.stack([g2, hr], axis=0)              # [t, s, p, l]
        g2all = np.ascontiguousarray(
            ga.transpose(2, 1, 0, 3)).reshape(P, S * 2 * P)
        in_c.append({"g2all": g2all, "indt": np.asarray(st["indt"])})
    res_c = bass_utils.run_bass_kernel_spmd(
        ncc, in_c, core_ids=list(range(N_CORES)), trace=trace)

    out = np.zeros((N, 1), dtype=np.float32)
    for c in range(N_CORES):
        ov = res_c.results[c]["out"].reshape(-1)
        real = st["perms"][c] >= 0
        out[st["perms"][c][real], 0] = ov[real]

    ts = [r.exec_time_ns for r in (res_a, res_b, res_c)]
    LAST_EXEC_NS = sum(t for t in ts if t) if any(ts) else None
    LAST_RESULTS = (res_a, res_b, res_c)
    return out


# revision 18
# speedup vs baseline: 1.0187x; 1.0023x over previous
"""Bass/Trainium2 kernel for 2-layer GAT (nn_GATa_45260365365735).

Three-launch payload-gather design (8 NeuronCores, SPMD):

  Launch A (node payloads): nodes are range-sharded across cores; each core
    computes PN = x @ wsc for its 12.5k nodes, where wsc [128, 12] packs the
    layer-1 linear algebra collapsed onto the attention vectors:
      cols 0:4  = e_src head logits   (W1 contracted with att_src1)
      cols 4:8  = z     head values   (W1 contracted with W2 — by linearity
                                       layer 2 only consumes h1 @ W2)
      cols 8:12 = e_dst head logits   (W1 contracted with att_dst1)
    Per-edge work therefore needs just 12 values per endpoint instead of the
    128-float feature row, cutting slot DMA ~10x vs gathering x[src].

  Host gathers PN into slot order (pure indexing / replication, as the
  baseline did with x[src]): the slot grid is TRANSPOSED — edge chunks on
  partitions, the 128 lanes (nodes) of a block on the free dim — packed
  densely into S stacks of 128 chunk-rows (blocks may straddle stacks).

  Launch B (layer 1): w = exp(leaky(e_src + e_dst)) per slot-head computed as
    max(exp(u), exp(0.2u)) (exp is monotone, so this IS exp(leaky(u)));
    wz = w*z; the per-destination segment sums become MATMULs with 0/1
    block-indicator stationary matrices (contraction over the chunk partition
    dim), accumulated across stacks into PSUM [NB, 512].  Epilogue:
    h2[d] = sum_h num/den + b1@W2 -> [NB, 128].

  Launch C (layer 2): host gathers h2[src]/h2[dst] into the same slot grid
    (scalar payloads); identical masked-softmax-reduce with heads=1.

  Padded slots ship e_src = -1e4 so exp() underflows to exactly 0 and they
  drop out of numerator and denominator; fully-padded lanes divide 0/0 and
  are discarded by the host inverse-permutation.  The reference's +1e-16 on
  the denominator is skipped: every real lane's denominator >= exp(leaky(
  self-loop logit)) >> 1e-16, so the epsilon is numerically invisible.
"""

import os
import numpy as np
import ml_dtypes

P = 128
N_CORES = 8
HEADS = 4
HID = 32
IN_DIM = 128
NEG_SLOPE = 0.2
KILL = -1.0e4
NMM = 512          # matmul moving free dim (psum bank f32 capacity)

_COMPILED = {}
LAST_EXEC_NS = None
LAST_RESULTS = None


# --------------------------------------------------------------------------
# host preprocessing (indexing / layout / param folding only)
# --------------------------------------------------------------------------

def _structure(edge_index, N):
    """Everything derivable from the graph structure alone."""
    ei = np.asarray(edge_index).astype(np.int64)
    src = np.concatenate([ei[0], np.arange(N, dtype=np.int64)])
    dst = np.concatenate([ei[1], np.arange(N, dtype=np.int64)])
    ET = src.shape[0]

    deg = np.bincount(dst, minlength=N).astype(np.int64)        # >= 1
    order = np.argsort(dst, kind="stable")
    src_sorted = src[order].astype(np.int32)
    estart = np.concatenate([[0], np.cumsum(deg)]).astype(np.int64)

    # round-robin by degree rank -> near-identical degree profiles per core
    grank = np.argsort(-deg, kind="stable")
    per = (N + N_CORES - 1) // N_CORES
    LP = int(np.ceil(per / P) * P)
    NB = LP // P
    perms = []
    for c in range(N_CORES):
        p = grank[c::N_CORES]
        perms.append(np.concatenate(
            [p, np.full(LP - len(p), -1, dtype=np.int64)]))

    CB = np.zeros(NB, dtype=np.int64)
    for c in range(N_CORES):
        d = np.where(perms[c] >= 0, deg[np.maximum(perms[c], 0)], 0)
        CB = np.maximum(CB, d.reshape(NB, P).max(axis=1))
    CB = np.maximum(CB, 1)
    offs = np.concatenate([[0], np.cumsum(CB)]).astype(np.int64)
    T1 = int(offs[-1])
    S = (T1 + P - 1) // P
    R = S * P

    # dense row packing: global chunk-row r belongs to block rowblk[r]
    rowblk = np.full(R, -1, dtype=np.int64)
    for b in range(NB):
        rowblk[offs[b]:offs[b + 1]] = b
    rowchunk = np.arange(R, dtype=np.int64) - np.where(
        rowblk >= 0, offs[np.maximum(rowblk, 0)], 0)

    # per-core slot grids [R, P]: src node id per slot (N = pad)
    lanes = np.arange(P, dtype=np.int64)[None, :]
    srcgrids, dstgrids = [], []
    for c in range(N_CORES):
        pids = perms[c]
        nd = np.where(rowblk[:, None] >= 0,
                      pids[np.maximum(rowblk[:, None], 0) * P + lanes], -1)
        ch = rowchunk[:, None]
        valid = (nd >= 0) & (ch < deg[np.maximum(nd, 0)]) & (rowblk[:, None] >= 0)
        eidx = np.clip(estart[np.maximum(nd, 0)] + ch, 0, ET - 1)
        sg = np.where(valid, src_sorted[eidx].astype(np.int64), N)
        dg = np.where(nd >= 0, nd, N)
        srcgrids.append(sg.astype(np.int32))
        dstgrids.append(dg.astype(np.int32))

    # per-stack block indicator [P, S*NB] (lhsT for segment-sum matmuls)
    indt = np.zeros((P, S * NB), dtype=np.float32)
    for r in range(R):
        b = rowblk[r]
        if b >= 0:
            indt[r % P, (r // P) * NB + b] = 1.0
    indt = indt.astype(ml_dtypes.bfloat16)

    return dict(N=N, LP=LP, NB=NB, T1=T1, S=S, R=R,
                perms=perms, srcgrids=srcgrids, dstgrids=dstgrids,
                indt=indt)


def _fold_params(W1, att_src1, att_dst1, b1, W2, att_src2, att_dst2, b2):
    W1 = np.asarray(W1, dtype=np.float32)
    a_s1 = np.asarray(att_src1, dtype=np.float32)
    a_d1 = np.asarray(att_dst1, dtype=np.float32)
    W2v = np.asarray(W2, dtype=np.float32).reshape(-1)
    W1r = W1.reshape(IN_DIM, HEADS, HID)
    W1a = np.einsum("khc,hc->kh", W1r, a_s1)
    W12 = np.einsum("khf,hf->kh", W1r, W2v.reshape(HEADS, HID))
    W1b = np.einsum("khc,hc->kh", W1r, a_d1)
    wsc = np.concatenate([W1a, W12, W1b], axis=1).astype(ml_dtypes.bfloat16)
    c0 = float(np.asarray(b1, dtype=np.float32).reshape(-1) @ W2v)
    as2 = float(np.asarray(att_src2).reshape(-1)[0])
    ad2 = float(np.asarray(att_dst2).reshape(-1)[0])
    b2f = float(np.asarray(b2).reshape(-1)[0])
    if abs(as2) < 1e-12:
        as2 = 1e-12 if as2 >= 0 else -1e-12
    return wsc, c0, as2, ad2, b2f


# --------------------------------------------------------------------------
# launch A: PN = x @ wsc for a contiguous node range
# --------------------------------------------------------------------------

def _build_a(NAP):
    from contextlib import ExitStack
    import concourse.tile as tile
    from concourse import bacc, mybir

    f32, bf16 = mybir.dt.float32, mybir.dt.bfloat16
    nc = bacc.Bacc("TRN2", target_bir_lowering=False, debug=False,
                   enable_asserts=False, num_devices=N_CORES)
    t_xt = nc.dram_tensor("xt", [P, NAP], bf16, kind="ExternalInput")
    t_wsc = nc.dram_tensor("wsc", [P, 12], bf16, kind="ExternalInput")
    t_pn = nc.dram_tensor("pn", [12, NAP], bf16, kind="ExternalOutput")

    n_mm = NAP // NMM
    CHK = 5 * NMM                   # x chunk: 5 matmuls' worth, in-order DMA
    with tile.TileContext(nc) as tc, ExitStack() as ctx:
        sb = ctx.enter_context(tc.tile_pool(name="sb", bufs=1))
        ps = ctx.enter_context(tc.tile_pool(name="ps", bufs=4, space="PSUM"))
        wsct = sb.tile([P, 12], bf16)
        nc.scalar.dma_start(wsct[:], t_wsc.ap())
        xt = sb.tile([P, NAP], bf16)
        # first chunk small so the matmul stream starts early; in-order on sync
        bounds = [0, 2 * NMM]
        while bounds[-1] < NAP:
            bounds.append(min(bounds[-1] + CHK, NAP))
        for o, e in zip(bounds[:-1], bounds[1:]):
            nc.sync.dma_start(xt[:, o:e], t_xt.ap()[:, o:e])
        po = sb.tile([12, NAP], bf16)
        # out-DMA groups of copied chunks; last groups small to shorten the tail
        obounds = list(range(0, max(n_mm - 5, 0), 5)) + \
            [n_mm - 5, n_mm - 3, n_mm - 1, n_mm]
        obounds = sorted(set(b for b in obounds if 0 <= b <= n_mm))
        fire = {e - 1: (s, e) for s, e in zip(obounds[:-1], obounds[1:])}
        for k in range(n_mm):
            pk = ps.tile([12, NMM], f32, tag="pk", name=f"pk{k}")
            nc.tensor.matmul(pk[:], lhsT=wsct[:],
                             rhs=xt[:, k * NMM:(k + 1) * NMM],
                             start=True, stop=True)
            dst = po[:, k * NMM:(k + 1) * NMM]
            if k % 2 == 0:
                nc.vector.tensor_copy(dst, pk[:])
            else:
                nc.scalar.copy(dst, pk[:])
            if k in fire:
                s_, e_ = fire[k]
                nc.sync.dma_start(t_pn.ap()[:, s_ * NMM:e_ * NMM],
                                  po[:, s_ * NMM:e_ * NMM])
    nc.compile()
    return nc


# --------------------------------------------------------------------------
# launch B: slot payloads -> per-node h2  (layer 1 + W2 collapse)
# --------------------------------------------------------------------------

def _build_b(S, NB, c0):
    from contextlib import ExitStack
    import concourse.tile as tile
    from concourse import bacc, mybir

    f32, bf16 = mybir.dt.float32, mybir.dt.bfloat16
    W = 4 * P                       # 4 head planes of 128 lanes
    nc = bacc.Bacc("TRN2", target_bir_lowering=False, debug=False,
                   enable_asserts=False, num_devices=N_CORES)
    t_g = nc.dram_tensor("gall", [P, S * 3 * W], bf16, kind="ExternalInput")
    t_ind = nc.dram_tensor("indt", [P, S * NB], bf16, kind="ExternalInput")
    t_h2 = nc.dram_tensor("h2", [NB, P], f32, kind="ExternalOutput")

    with tile.TileContext(nc) as tc, ExitStack() as ctx:
        sb = ctx.enter_context(tc.tile_pool(name="sb", bufs=1))
        sp = ctx.enter_context(tc.tile_pool(name="sp", bufs=3))
        ps = ctx.enter_context(tc.tile_pool(name="ps", bufs=2, space="PSUM"))

        # one DMA head: queue FIFO => arrival follows issue order.  Stack 0
        # first so compute starts ASAP; indicator needed only by the matmuls.
        g = sb.tile([P, S * 3 * W], bf16)
        indt = sb.tile([P, S * NB], bf16)

        def load_stack(s):
            sl = slice(s * 3 * W, (s + 1) * 3 * W)
            nc.sync.dma_start(g[:, sl], t_g.ap()[:, sl])
        load_stack(0)
        nc.sync.dma_start(indt[:], t_ind.ap())
        for s in range(1, S):
            load_stack(s)

        num = ps.tile([NB, W], f32, tag="num")
        den = ps.tile([NB, W], f32, tag="den")

        # engine balance: most stacks compute exp(leaky(u)) as
        # max(exp(u), exp(0.2u)) (2 scalar ACTs, 1 vector max); one stack
        # uses vector leaky + single exp to even out vector/scalar load.
        vpath = {S - 2} if S >= 2 else set()
        for s in range(S):
            base = s * 3 * W
            ge = g[:, base:base + W]
            ed = g[:, base + W:base + 2 * W]
            gz = g[:, base + 2 * W:base + 3 * W]
            u = sp.tile([P, W], bf16, tag="u", name=f"u{s}")
            nc.vector.tensor_tensor(out=u[:], in0=ge, in1=ed,
                                    op=mybir.AluOpType.add)
            w = sp.tile([P, W], bf16, tag="w", name=f"w{s}")
            if s in vpath:
                lr = sp.tile([P, W], bf16, tag="lr", name=f"lr{s}")
                nc.vector.scalar_tensor_tensor(
                    out=lr[:], in0=u[:], scalar=NEG_SLOPE, in1=u[:],
                    op0=mybir.AluOpType.mult, op1=mybir.AluOpType.max)
                nc.scalar.activation(w[:], lr[:],
                                     mybir.ActivationFunctionType.Exp)
            else:
                e1 = sp.tile([P, W], bf16, tag="e1", name=f"e1{s}")
                nc.scalar.activation(e1[:], u[:],
                                     mybir.ActivationFunctionType.Exp)
                e2 = sp.tile([P, W], bf16, tag="e2", name=f"e2{s}")
                nc.scalar.activation(e2[:], u[:],
                                     mybir.ActivationFunctionType.Exp,
                                     scale=NEG_SLOPE)
                nc.vector.tensor_tensor(out=w[:], in0=e1[:], in1=e2[:],
                                        op=mybir.AluOpType.max)
            lhs = indt[:, s * NB:(s + 1) * NB]
            nc.tensor.matmul(den[:], lhsT=lhs, rhs=w[:],
                             start=(s == 0), stop=(s == S - 1))
            wz = sp.tile([P, W], bf16, tag="wz", name=f"wz{s}")
            nc.vector.tensor_tensor(out=wz[:], in0=w[:], in1=gz,
                                    op=mybir.AluOpType.mult)
            nc.tensor.matmul(num[:], lhsT=lhs, rhs=wz[:],
                             start=(s == 0), stop=(s == S - 1))

        rc = sb.tile([NB, W], f32)
        nc.vector.reciprocal_approx_fast(out=rc[:], in_=den[:])
        nr = sb.tile([NB, W], f32)
        nc.vector.tensor_tensor(out=nr[:], in0=num[:], in1=rc[:],
                                op=mybir.AluOpType.mult)
        # h2 = sum over the 4 head planes (contiguous slices beat a strided
        # tensor_reduce here)
        t01 = sb.tile([NB, P], f32)
        nc.vector.tensor_tensor(out=t01[:], in0=nr[:, 0:P], in1=nr[:, P:2 * P],
                                op=mybir.AluOpType.add)
        t23 = sb.tile([NB, P], f32)
        nc.vector.tensor_tensor(out=t23[:], in0=nr[:, 2 * P:3 * P],
                                in1=nr[:, 3 * P:4 * P], op=mybir.AluOpType.add)
        h2 = sb.tile([NB, P], f32)
        nc.vector.tensor_tensor(out=h2[:], in0=t01[:], in1=t23[:],
                                op=mybir.AluOpType.add)
        if c0 != 0.0:
            h2o = sb.tile([NB, P], f32)
            nc.vector.tensor_scalar(h2o[:], h2[:], c0, None,
                                    op0=mybir.AluOpType.add)
            h2 = h2o
        nc.sync.dma_start(t_h2.ap()[:], h2[:])
    nc.compile()
    return nc


# --------------------------------------------------------------------------
# launch C: h2 slot payloads -> output  (layer 2, heads=1)
# --------------------------------------------------------------------------

def _build_c(S, NB, as2, ad2, b2f):
    from contextlib import ExitStack
    import concourse.tile as tile
    from concourse import bacc, mybir

    f32, bf16 = mybir.dt.float32, mybir.dt.bfloat16
    nc = bacc.Bacc("TRN2", target_bir_lowering=False, debug=False,
                   enable_asserts=False, num_devices=N_CORES)
    t_g = nc.dram_tensor("g2all", [P, S * 2 * P], bf16, kind="ExternalInput")
    t_ind = nc.dram_tensor("indt", [P, S * NB], bf16, kind="ExternalInput")
    t_out = nc.dram_tensor("out", [NB, P], f32, kind="ExternalOutput")

    ratio = ad2 / as2
    # split stacks into 2 pipeline chunks
    half = (S + 1) // 2
    chunks = [(0, half), (half, S)] if S > 1 else [(0, S)]
    with tile.TileContext(nc) as tc, ExitStack() as ctx:
        sb = ctx.enter_context(tc.tile_pool(name="sb", bufs=1))
        ps = ctx.enter_context(tc.tile_pool(name="ps", bufs=2, space="PSUM"))

        indt = sb.tile([P, S * NB], bf16)
        g = sb.tile([P, S * 2 * P], bf16)
        nc.sync.dma_start(g[:, 0:chunks[0][1] * 2 * P],
                          t_g.ap()[:, 0:chunks[0][1] * 2 * P])
        nc.sync.dma_start(indt[:], t_ind.ap())
        for (s0, s1) in chunks[1:]:
            nc.sync.dma_start(g[:, s0 * 2 * P:s1 * 2 * P],
                              t_g.ap()[:, s0 * 2 * P:s1 * 2 * P])

        num = ps.tile([NB, P], f32, tag="num")
        den = ps.tile([NB, P], f32, tag="den")
        w = sb.tile([P, S * P], bf16)
        wg = sb.tile([P, S * P], bf16)

        first = True
        for (s0, s1) in chunks:
            n = s1 - s0
            gv = g[:, s0 * 2 * P:s1 * 2 * P].rearrange(
                "p (s t l) -> p s t l", s=n, t=2, l=P)
            g2 = gv[:, :, 0, :]
            hr = gv[:, :, 1, :]
            v = sb.tile([P, n * P], f32, name=f"v{s0}")
            v3 = v[:].rearrange("p (s l) -> p s l", s=n, l=P)
            nc.vector.scalar_tensor_tensor(
                out=v3, in0=hr, scalar=ratio, in1=g2,
                op0=mybir.AluOpType.mult, op1=mybir.AluOpType.add)
            e1 = sb.tile([P, n * P], bf16, name=f"e1{s0}")
            nc.scalar.activation(e1[:], v[:],
                                 mybir.ActivationFunctionType.Exp, scale=as2)
            e2 = sb.tile([P, n * P], bf16, name=f"e2{s0}")
            nc.scalar.activation(e2[:], v[:],
                                 mybir.ActivationFunctionType.Exp,
                                 scale=as2 * NEG_SLOPE)
            wv = w[:, s0 * P:s1 * P]
            nc.vector.tensor_tensor(out=wv, in0=e1[:], in1=e2[:],
                                    op=mybir.AluOpType.max)
            wgv = wg[:, s0 * P:s1 * P].rearrange("p (s l) -> p s l", s=n, l=P)
            nc.vector.tensor_tensor(
                out=wgv, in0=w[:, s0 * P:s1 * P].rearrange(
                    "p (s l) -> p s l", s=n, l=P),
                in1=g2, op=mybir.AluOpType.mult)
            for s in range(s0, s1):
                lhs = indt[:, s * NB:(s + 1) * NB]
                nc.tensor.matmul(den[:], lhsT=lhs,
                                 rhs=w[:, s * P:(s + 1) * P],
                                 start=(s == 0), stop=(s == S - 1))
                nc.tensor.matmul(num[:], lhsT=lhs,
                                 rhs=wg[:, s * P:(s + 1) * P],
                                 start=(s == 0), stop=(s == S - 1))
            first = False

        rc = sb.tile([NB, P], f32)
        nc.vector.reciprocal_approx_fast(out=rc[:], in_=den[:])
        o = sb.tile([NB, P], f32)
        nc.vector.tensor_tensor(out=o[:], in0=num[:], in1=rc[:],
                                op=mybir.AluOpType.mult)
        if b2f != 0.0:
            ob = sb.tile([NB, P], f32)
            nc.vector.tensor_scalar(ob[:], o[:], b2f, None,
                                    op0=mybir.AluOpType.add)
            o = ob
        nc.sync.dma_start(t_out.ap()[:], o[:])
    nc.compile()
    return nc


# --------------------------------------------------------------------------
# entry point
# --------------------------------------------------------------------------

def _install_ntff_shim():
    """Optional: register the axon NTFF profiling hook (dev tracing only)."""
    import sys as _sys
    import types as _types
    if "antenv.axon_hooks" in _sys.modules:
        return
    try:
        import antenv
        mod = _types.ModuleType("antenv.axon_hooks")
        _state = {"hook": None}
        mod.set_axon_ntff_profile_hook = lambda h: _state.__setitem__("hook", h)
        mod.get_axon_ntff_profile_hook = lambda: _state["hook"]
        _sys.modules["antenv.axon_hooks"] = mod
        antenv.axon_hooks = mod
        from trn_agent_boot.trn_boot import _ntff_profile_via_ctypes
        mod.set_axon_ntff_profile_hook(
            _ntff_profile_via_ctypes("/opt/axon/libaxon_pjrt.so"))
    except Exception as e:  # pragma: no cover
        print("ntff shim unavailable:", e)


def kernel(**inputs):
    global LAST_EXEC_NS, LAST_RESULTS
    from concourse import bass_utils

    x = np.asarray(inputs["x"], dtype=np.float32)
    N = x.shape[0]
    st = _structure(inputs["edge_index"], N)
    wsc, c0, as2, ad2, b2f = _fold_params(
        inputs["W1"], inputs["att_src1"], inputs["att_dst1"], inputs["b1"],
        inputs["W2"], inputs["att_src2"], inputs["att_dst2"], inputs["b2"])

    S, NB, R, LP = st["S"], st["NB"], st["R"], st["LP"]
    per = (N + N_CORES - 1) // N_CORES
    NAP = ((per + NMM - 1) // NMM) * NMM

    key = (N, S, NB, st["T1"], round(c0, 9), round(as2, 12),
           round(ad2, 12), round(b2f, 9))
    if key not in _COMPILED:
        _COMPILED[key] = (_build_a(NAP), _build_b(S, NB, c0),
                          _build_c(S, NB, as2, ad2, b2f))
    nca, ncb, ncc = _COMPILED[key]

    trace = os.environ.get("GAT_TRACE", "0") == "1"
    if trace:
        _install_ntff_shim()

    # ---- launch A
    xbf = x.astype(ml_dtypes.bfloat16)
    in_a = []
    for c in range(N_CORES):
        lo = c * per
        xt = np.zeros((P, NAP), dtype=ml_dtypes.bfloat16)
        n_c = min(per, N - lo)
        xt[:, :n_c] = xbf[lo:lo + n_c].T
        in_a.append({"xt": xt, "wsc": np.asarray(wsc)})
    res_a = bass_utils.run_bass_kernel_spmd(
        nca, in_a, core_ids=list(range(N_CORES)), trace=trace)

    # host: assemble padded payload table [12, N+1] (col N = padding)
    pn_bf = np.zeros((12, N + 1), dtype=ml_dtypes.bfloat16)
    for c in range(N_CORES):
        lo = c * per
        n_c = min(per, N - lo)
        pn_bf[:, lo:lo + n_c] = res_a.results[c]["pn"][:, :n_c]
    pn_bf[0:4, N] = KILL

    # ---- launch B inputs: gather payloads into slot grids
    in_b = []
    for c in range(N_CORES):
        sub = pn_bf[:, st["srcgrids"][c]]            # [12, R, 128]
        edr = pn_bf[8:12][:, st["dstgrids"][c]]      # [4, R, 128]
        big = np.stack([sub[0:4], edr, sub[4:8]], axis=0)   # [sec, h, R, l]
        big = big.reshape(3, 4, S, P, P)
        gall = np.ascontiguousarray(
            big.transpose(3, 2, 0, 1, 4)).reshape(P, S * 3 * 4 * P)
        in_b.append({"gall": gall, "indt": np.asarray(st["indt"])})
    res_b = bass_utils.run_bass_kernel_spmd(
        ncb, in_b, core_ids=list(range(N_CORES)), trace=trace)

    # host: scatter h2 back to node order, with kill/zero padded tables
    h2_node = np.zeros(N, dtype=np.float32)
    for c in range(N_CORES):
        h2v = res_b.results[c]["h2"].reshape(-1)     # [NB*P] block-major
        real = st["perms"][c] >= 0
        h2_node[st["perms"][c][real]] = h2v[real]
    h2_kill = np.zeros(N + 1, dtype=np.float32)
    h2_kill[:N] = h2_node
    h2_kill[N] = 2.0 * KILL / as2
    h2_zero = np.zeros(N + 1, dtype=np.float32)
    h2_zero[:N] = h2_node
    h2k_bf = h2_kill.astype(ml_dtypes.bfloat16)
    h2z_bf = h2_zero.astype(ml_dtypes.bfloat16)

    # ---- launch C inputs
    in_c = []
    for c in range(N_CORES):
        g2 = h2k_bf[st["srcgrids"][c]].reshape(S, P, P)
        hr = h2z_bf[st["dstgrids"][c]].reshape(S, P, P)
        ga = np.stack([g2, hr], axis=0)              # [t, s, p, l]
        g2all = np.ascontiguousarray(
            ga.transpose(2, 1, 0, 3)).reshape(P, S * 2 * P)
        in_c.append({"g2all": g2all, "indt": np.asarray(st["indt"])})
    res_c = bass_utils.run_bass_kernel_spmd(
        ncc, in_c, core_ids=list(range(N_CORES)), trace=trace)

    out = np.zeros((N, 1), dtype=np.float32)
    for c in range(N_CORES):
        ov = res_c.results[c]["out"].reshape(-1)
        real = st["perms"][c] >= 0
        out[st["perms"][c][real], 0] = ov[real]

    ts = [r.exec_time_ns for r in (res_a, res_b, res_c)]
    LAST_EXEC_NS = sum(t for t in ts if t) if any(ts) else None
    LAST_RESULTS = (res_a, res_b, res_c)
    return out


# revision 20
# speedup vs baseline: 1.0253x; 1.0065x over previous
"""Bass/Trainium2 kernel for 2-layer GAT (nn_GATa_45260365365735).

Three-launch payload-gather design (8 NeuronCores, SPMD):

  Launch A (node payloads): nodes are range-sharded across cores; each core
    computes PN = x @ wsc for its 12.5k nodes, where wsc [128, 12] packs the
    layer-1 linear algebra collapsed onto the attention vectors:
      cols 0:4  = e_src head logits   (W1 contracted with att_src1)
      cols 4:8  = z     head values   (W1 contracted with W2 — by linearity
                                       layer 2 only consumes h1 @ W2)
      cols 8:12 = e_dst head logits   (W1 contracted with att_dst1)
    Per-edge work therefore needs just 12 values per endpoint instead of the
    128-float feature row, cutting slot DMA ~10x vs gathering x[src].

  Host gathers PN into slot order (pure indexing / replication, as the
  baseline did with x[src]): the slot grid is TRANSPOSED — edge chunks on
  partitions, the 128 lanes (nodes) of a block on the free dim — packed
  densely into S stacks of 128 chunk-rows (blocks may straddle stacks).

  Launch B (layer 1): w = exp(leaky(e_src + e_dst)) per slot-head computed as
    max(exp(u), exp(0.2u)) (exp is monotone, so this IS exp(leaky(u)));
    wz = w*z; the per-destination segment sums become MATMULs with 0/1
    block-indicator stationary matrices (contraction over the chunk partition
    dim), accumulated across stacks into PSUM [NB, 512].  Epilogue:
    h2[d] = sum_h num/den + b1@W2 -> [NB, 128].

  Launch C (layer 2): host gathers h2[src]/h2[dst] into the same slot grid
    (scalar payloads); identical masked-softmax-reduce with heads=1.

  Padded slots ship e_src = -1e4 so exp() underflows to exactly 0 and they
  drop out of numerator and denominator; fully-padded lanes divide 0/0 and
  are discarded by the host inverse-permutation.  The reference's +1e-16 on
  the denominator is skipped: every real lane's denominator >= exp(leaky(
  self-loop logit)) >> 1e-16, so the epsilon is numerically invisible.
"""

import os
import numpy as np
import ml_dtypes

P = 128
N_CORES = 8
HEADS = 4
HID = 32
IN_DIM = 128
NEG_SLOPE = 0.2
KILL = -1.0e4
NMM = 512          # matmul moving free dim (psum bank f32 capacity)

_COMPILED = {}
LAST_EXEC_NS = None
LAST_RESULTS = None


# --------------------------------------------------------------------------
# host preprocessing (indexing / layout / param folding only)
# --------------------------------------------------------------------------

def _structure(edge_index, N):
    """Everything derivable from the graph structure alone."""
    ei = np.asarray(edge_index).astype(np.int64)
    src = np.concatenate([ei[0], np.arange(N, dtype=np.int64)])
    dst = np.concatenate([ei[1], np.arange(N, dtype=np.int64)])
    ET = src.shape[0]

    deg = np.bincount(dst, minlength=N).astype(np.int64)        # >= 1
    order = np.argsort(dst, kind="stable")
    src_sorted = src[order].astype(np.int32)
    estart = np.concatenate([[0], np.cumsum(deg)]).astype(np.int64)

    # round-robin by degree rank -> near-identical degree profiles per core
    grank = np.argsort(-deg, kind="stable")
    per = (N + N_CORES - 1) // N_CORES
    LP = int(np.ceil(per / P) * P)
    NB = LP // P
    perms = []
    for c in range(N_CORES):
        p = grank[c::N_CORES]
        perms.append(np.concatenate(
            [p, np.full(LP - len(p), -1, dtype=np.int64)]))

    CB = np.zeros(NB, dtype=np.int64)
    for c in range(N_CORES):
        d = np.where(perms[c] >= 0, deg[np.maximum(perms[c], 0)], 0)
        CB = np.maximum(CB, d.reshape(NB, P).max(axis=1))
    CB = np.maximum(CB, 1)
    offs = np.concatenate([[0], np.cumsum(CB)]).astype(np.int64)
    T1 = int(offs[-1])
    S = (T1 + P - 1) // P
    R = S * P

    # dense row packing: global chunk-row r belongs to block rowblk[r]
    rowblk = np.full(R, -1, dtype=np.int64)
    for b in range(NB):
        rowblk[offs[b]:offs[b + 1]] = b
    rowchunk = np.arange(R, dtype=np.int64) - np.where(
        rowblk >= 0, offs[np.maximum(rowblk, 0)], 0)

    # per-core slot grids [R, P]: src node id per slot (N = pad)
    lanes = np.arange(P, dtype=np.int64)[None, :]
    srcgrids, dstgrids = [], []
    for c in range(N_CORES):
        pids = perms[c]
        nd = np.where(rowblk[:, None] >= 0,
                      pids[np.maximum(rowblk[:, None], 0) * P + lanes], -1)
        ch = rowchunk[:, None]
        valid = (nd >= 0) & (ch < deg[np.maximum(nd, 0)]) & (rowblk[:, None] >= 0)
        eidx = np.clip(estart[np.maximum(nd, 0)] + ch, 0, ET - 1)
        sg = np.where(valid, src_sorted[eidx].astype(np.int64), N)
        dg = np.where(nd >= 0, nd, N)
        srcgrids.append(sg.astype(np.int32))
        dstgrids.append(dg.astype(np.int32))

    # per-stack block indicator [P, S*NB] (lhsT for segment-sum matmuls)
    indt = np.zeros((P, S * NB), dtype=np.float32)
    for r in range(R):
        b = rowblk[r]
        if b >= 0:
            indt[r % P, (r // P) * NB + b] = 1.0
    indt = indt.astype(ml_dtypes.bfloat16)

    return dict(N=N, LP=LP, NB=NB, T1=T1, S=S, R=R,
                perms=perms, srcgrids=srcgrids, dstgrids=dstgrids,
                indt=indt)


def _fold_params(W1, att_src1, att_dst1, b1, W2, att_src2, att_dst2, b2):
    W1 = np.asarray(W1, dtype=np.float32)
    a_s1 = np.asarray(att_src1, dtype=np.float32)
    a_d1 = np.asarray(att_dst1, dtype=np.float32)
    W2v = np.asarray(W2, dtype=np.float32).reshape(-1)
    W1r = W1.reshape(IN_DIM, HEADS, HID)
    W1a = np.einsum("khc,hc->kh", W1r, a_s1)
    W12 = np.einsum("khf,hf->kh", W1r, W2v.reshape(HEADS, HID))
    W1b = np.einsum("khc,hc->kh", W1r, a_d1)
    wsc = np.concatenate([W1a, W12, W1b], axis=1).astype(ml_dtypes.bfloat16)
    c0 = float(np.asarray(b1, dtype=np.float32).reshape(-1) @ W2v)
    as2 = float(np.asarray(att_src2).reshape(-1)[0])
    ad2 = float(np.asarray(att_dst2).reshape(-1)[0])
    b2f = float(np.asarray(b2).reshape(-1)[0])
    if abs(as2) < 1e-12:
        as2 = 1e-12 if as2 >= 0 else -1e-12
    return wsc, c0, as2, ad2, b2f


# --------------------------------------------------------------------------
# launch A: PN = x @ wsc for a contiguous node range
# --------------------------------------------------------------------------

def _build_a(NAP):
    from contextlib import ExitStack
    import concourse.tile as tile
    from concourse import bacc, mybir

    f32, bf16 = mybir.dt.float32, mybir.dt.bfloat16
    nc = bacc.Bacc("TRN2", target_bir_lowering=False, debug=False,
                   enable_asserts=False, num_devices=N_CORES)
    t_xt = nc.dram_tensor("xt", [P, NAP], bf16, kind="ExternalInput")
    t_wsc = nc.dram_tensor("wsc", [P, 12], bf16, kind="ExternalInput")
    t_pn = nc.dram_tensor("pn", [12, NAP], bf16, kind="ExternalOutput")

    n_mm = NAP // NMM
    CHK = 5 * NMM                   # x chunk: 5 matmuls' worth, in-order DMA
    with tile.TileContext(nc) as tc, ExitStack() as ctx:
        sb = ctx.enter_context(tc.tile_pool(name="sb", bufs=1))
        ps = ctx.enter_context(tc.tile_pool(name="ps", bufs=4, space="PSUM"))
        wsct = sb.tile([P, 12], bf16)
        nc.scalar.dma_start(wsct[:], t_wsc.ap())
        xt = sb.tile([P, NAP], bf16)
        # first chunk small so the matmul stream starts early; in-order on sync
        bounds = [0, 2 * NMM]
        while bounds[-1] < NAP:
            bounds.append(min(bounds[-1] + CHK, NAP))
        for o, e in zip(bounds[:-1], bounds[1:]):
            nc.sync.dma_start(xt[:, o:e], t_xt.ap()[:, o:e])
        po = sb.tile([12, NAP], bf16)
        # out-DMA groups of copied chunks; last groups small to shorten the tail
        obounds = list(range(0, max(n_mm - 5, 0), 5)) + \
            [n_mm - 5, n_mm - 3, n_mm - 1, n_mm]
        obounds = sorted(set(b for b in obounds if 0 <= b <= n_mm))
        fire = {e - 1: (s, e) for s, e in zip(obounds[:-1], obounds[1:])}
        for k in range(n_mm):
            pk = ps.tile([12, NMM], f32, tag="pk", name=f"pk{k}")
            nc.tensor.matmul(pk[:], lhsT=wsct[:],
                             rhs=xt[:, k * NMM:(k + 1) * NMM],
                             start=True, stop=True)
            dst = po[:, k * NMM:(k + 1) * NMM]
            if k % 2 == 0:
                nc.vector.tensor_copy(dst, pk[:])
            else:
                nc.scalar.copy(dst, pk[:])
            if k in fire:
                s_, e_ = fire[k]
                nc.sync.dma_start(t_pn.ap()[:, s_ * NMM:e_ * NMM],
                                  po[:, s_ * NMM:e_ * NMM])
    nc.compile()
    return nc


# --------------------------------------------------------------------------
# launch B: slot payloads -> per-node h2  (layer 1 + W2 collapse)
# --------------------------------------------------------------------------

def _build_b(S, NB, c0):
    from contextlib import ExitStack
    import concourse.tile as tile
    from concourse import bacc, mybir

    f32, bf16 = mybir.dt.float32, mybir.dt.bfloat16
    W = 4 * P                       # 4 head planes of 128 lanes
    nc = bacc.Bacc("TRN2", target_bir_lowering=False, debug=False,
                   enable_asserts=False, num_devices=N_CORES)
    t_g = nc.dram_tensor("gall", [P, S * 3 * W], bf16, kind="ExternalInput")
    t_ind = nc.dram_tensor("indt", [P, S * NB], bf16, kind="ExternalInput")
    t_h2 = nc.dram_tensor("h2", [NB, P], f32, kind="ExternalOutput")

    with tile.TileContext(nc) as tc, ExitStack() as ctx:
        sb = ctx.enter_context(tc.tile_pool(name="sb", bufs=1))
        sp = ctx.enter_context(tc.tile_pool(name="sp", bufs=3))
        ps = ctx.enter_context(tc.tile_pool(name="ps", bufs=2, space="PSUM"))

        # one DMA head: queue FIFO => arrival follows issue order.  Stack 0
        # first so compute starts ASAP; indicator needed only by the matmuls.
        g = sb.tile([P, S * 3 * W], bf16)
        indt = sb.tile([P, S * NB], bf16)

        nc.scalar.dma_start(indt[:], t_ind.ap())
        for s in range(S):
            sl = slice(s * 3 * W, (s + 1) * 3 * W)
            nc.sync.dma_start(g[:, sl], t_g.ap()[:, sl])

        num = ps.tile([NB, W], f32, tag="num")
        den = ps.tile([NB, W], f32, tag="den")

        # engine balance: most stacks compute exp(leaky(u)) as
        # max(exp(u), exp(0.2u)) (2 scalar ACTs, 1 vector max); one stack
        # uses vector leaky + single exp to even out vector/scalar load.
        vpath = {S - 2} if S >= 2 else set()
        for s in range(S):
            base = s * 3 * W
            ge = g[:, base:base + W]
            ed = g[:, base + W:base + 2 * W]
            gz = g[:, base + 2 * W:base + 3 * W]
            u = sp.tile([P, W], bf16, tag="u", name=f"u{s}")
            nc.vector.tensor_tensor(out=u[:], in0=ge, in1=ed,
                                    op=mybir.AluOpType.add)
            w = sp.tile([P, W], bf16, tag="w", name=f"w{s}")
            if s in vpath:
                lr = sp.tile([P, W], bf16, tag="lr", name=f"lr{s}")
                nc.vector.scalar_tensor_tensor(
                    out=lr[:], in0=u[:], scalar=NEG_SLOPE, in1=u[:],
                    op0=mybir.AluOpType.mult, op1=mybir.AluOpType.max)
                nc.scalar.activation(w[:], lr[:],
                                     mybir.ActivationFunctionType.Exp)
            else:
                e1 = sp.tile([P, W], bf16, tag="e1", name=f"e1{s}")
                nc.scalar.activation(e1[:], u[:],
                                     mybir.ActivationFunctionType.Exp)
                e2 = sp.tile([P, W], bf16, tag="e2", name=f"e2{s}")
                nc.scalar.activation(e2[:], u[:],
                                     mybir.ActivationFunctionType.Exp,
                                     scale=NEG_SLOPE)
                nc.vector.tensor_tensor(out=w[:], in0=e1[:], in1=e2[:],
                                        op=mybir.AluOpType.max)
            lhs = indt[:, s * NB:(s + 1) * NB]
            nc.tensor.matmul(den[:], lhsT=lhs, rhs=w[:],
                             start=(s == 0), stop=(s == S - 1))
            wz = sp.tile([P, W], bf16, tag="wz", name=f"wz{s}")
            nc.vector.tensor_tensor(out=wz[:], in0=w[:], in1=gz,
                                    op=mybir.AluOpType.mult)
            nc.tensor.matmul(num[:], lhsT=lhs, rhs=wz[:],
                             start=(s == 0), stop=(s == S - 1))

        rc = sb.tile([NB, W], f32)
        nc.vector.reciprocal_approx_fast(out=rc[:], in_=den[:])
        nr = sb.tile([NB, W], f32)
        nc.vector.tensor_tensor(out=nr[:], in0=num[:], in1=rc[:],
                                op=mybir.AluOpType.mult)
        # h2 = sum over the 4 head planes (contiguous slices beat a strided
        # tensor_reduce here)
        t01 = sb.tile([NB, P], f32)
        nc.vector.tensor_tensor(out=t01[:], in0=nr[:, 0:P], in1=nr[:, P:2 * P],
                                op=mybir.AluOpType.add)
        t23 = sb.tile([NB, P], f32)
        nc.vector.tensor_tensor(out=t23[:], in0=nr[:, 2 * P:3 * P],
                                in1=nr[:, 3 * P:4 * P], op=mybir.AluOpType.add)
        h2 = sb.tile([NB, P], f32)
        nc.vector.tensor_tensor(out=h2[:], in0=t01[:], in1=t23[:],
                                op=mybir.AluOpType.add)
        if c0 != 0.0:
            h2o = sb.tile([NB, P], f32)
            nc.vector.tensor_scalar(h2o[:], h2[:], c0, None,
                                    op0=mybir.AluOpType.add)
            h2 = h2o
        nc.sync.dma_start(t_h2.ap()[:], h2[:])
    nc.compile()
    return nc


# --------------------------------------------------------------------------
# launch C: h2 slot payloads -> output  (layer 2, heads=1)
# --------------------------------------------------------------------------

def _build_c(S, NB, as2, ad2, b2f):
    from contextlib import ExitStack
    import concourse.tile as tile
    from concourse import bacc, mybir

    f32, bf16 = mybir.dt.float32, mybir.dt.bfloat16
    nc = bacc.Bacc("TRN2", target_bir_lowering=False, debug=False,
                   enable_asserts=False, num_devices=N_CORES)
    t_g = nc.dram_tensor("g2all", [P, S * 2 * P], bf16, kind="ExternalInput")
    t_ind = nc.dram_tensor("indt", [P, S * NB], bf16, kind="ExternalInput")
    t_out = nc.dram_tensor("out", [NB, P], f32, kind="ExternalOutput")

    ratio = ad2 / as2
    # split stacks into 2 pipeline chunks
    half = (S + 1) // 2
    chunks = [(0, half), (half, S)] if S > 1 else [(0, S)]
    with tile.TileContext(nc) as tc, ExitStack() as ctx:
        sb = ctx.enter_context(tc.tile_pool(name="sb", bufs=1))
        ps = ctx.enter_context(tc.tile_pool(name="ps", bufs=2, space="PSUM"))

        indt = sb.tile([P, S * NB], bf16)
        nc.scalar.dma_start(indt[:], t_ind.ap())
        g = sb.tile([P, S * 2 * P], bf16)
        for (s0, s1) in chunks:
            nc.sync.dma_start(g[:, s0 * 2 * P:s1 * 2 * P],
                              t_g.ap()[:, s0 * 2 * P:s1 * 2 * P])

        num = ps.tile([NB, P], f32, tag="num")
        den = ps.tile([NB, P], f32, tag="den")
        w = sb.tile([P, S * P], bf16)
        wg = sb.tile([P, S * P], bf16)

        first = True
        for (s0, s1) in chunks:
            n = s1 - s0
            gv = g[:, s0 * 2 * P:s1 * 2 * P].rearrange(
                "p (s t l) -> p s t l", s=n, t=2, l=P)
            g2 = gv[:, :, 0, :]
            hr = gv[:, :, 1, :]
            v = sb.tile([P, n * P], f32, name=f"v{s0}")
            v3 = v[:].rearrange("p (s l) -> p s l", s=n, l=P)
            nc.vector.scalar_tensor_tensor(
                out=v3, in0=hr, scalar=ratio, in1=g2,
                op0=mybir.AluOpType.mult, op1=mybir.AluOpType.add)
            e1 = sb.tile([P, n * P], bf16, name=f"e1{s0}")
            nc.scalar.activation(e1[:], v[:],
                                 mybir.ActivationFunctionType.Exp, scale=as2)
            e2 = sb.tile([P, n * P], bf16, name=f"e2{s0}")
            nc.scalar.activation(e2[:], v[:],
                                 mybir.ActivationFunctionType.Exp,
                                 scale=as2 * NEG_SLOPE)
            wv = w[:, s0 * P:s1 * P]
            nc.vector.tensor_tensor(out=wv, in0=e1[:], in1=e2[:],
                                    op=mybir.AluOpType.max)
            wgv = wg[:, s0 * P:s1 * P].rearrange("p (s l) -> p s l", s=n, l=P)
            nc.vector.tensor_tensor(
                out=wgv, in0=w[:, s0 * P:s1 * P].rearrange(
                    "p (s l) -> p s l", s=n, l=P),
                in1=g2, op=mybir.AluOpType.mult)
            for s in range(s0, s1):
                lhs = indt[:, s * NB:(s + 1) * NB]
                nc.tensor.matmul(den[:], lhsT=lhs,
                                 rhs=w[:, s * P:(s + 1) * P],
                                 start=(s == 0), stop=(s == S - 1))
                nc.tensor.matmul(num[:], lhsT=lhs,
                                 rhs=wg[:, s * P:(s + 1) * P],
                                 start=(s == 0), stop=(s == S - 1))
            first = False

        rc = sb.tile([NB, P], f32)
        nc.vector.reciprocal_approx_fast(out=rc[:], in_=den[:])
        o = sb.tile([NB, P], f32)
        nc.vector.tensor_tensor(out=o[:], in0=num[:], in1=rc[:],
                                op=mybir.AluOpType.mult)
        if b2f != 0.0:
            ob = sb.tile([NB, P], f32)
            nc.vector.tensor_scalar(ob[:], o[:], b2f, None,
                                    op0=mybir.AluOpType.add)
            o = ob
        nc.sync.dma_start(t_out.ap()[:], o[:])
    nc.compile()
    return nc


# --------------------------------------------------------------------------
# entry point
# --------------------------------------------------------------------------

def _install_ntff_shim():
    """Optional: register the axon NTFF profiling hook (dev tracing only)."""
    import sys as _sys
    import types as _types
    if "antenv.axon_hooks" in _sys.modules:
        return
    try:
        import antenv
        mod = _types.ModuleType("antenv.axon_hooks")
        _state = {"hook": None}
        mod.set_axon_ntff_profile_hook = lambda h: _state.__setitem__("hook", h)
        mod.get_axon_ntff_profile_hook = lambda: _state["hook"]
        _sys.modules["antenv.axon_hooks"] = mod
        antenv.axon_hooks = mod
        from trn_agent_boot.trn_boot import _ntff_profile_via_ctypes
        mod.set_axon_ntff_profile_hook(
            _ntff_profile_via_ctypes("/opt/axon/libaxon_pjrt.so"))
    except Exception as e:  # pragma: no cover
        print("ntff shim unavailable:", e)


def kernel(**inputs):
    global LAST_EXEC_NS, LAST_RESULTS
    from concourse import bass_utils

    x = np.asarray(inputs["x"], dtype=np.float32)
    N = x.shape[0]
    st = _structure(inputs["edge_index"], N)
    wsc, c0, as2, ad2, b2f = _fold_params(
        inputs["W1"], inputs["att_src1"], inputs["att_dst1"], inputs["b1"],
        inputs["W2"], inputs["att_src2"], inputs["att_dst2"], inputs["b2"])

    S, NB, R, LP = st["S"], st["NB"], st["R"], st["LP"]
    per = (N + N_CORES - 1) // N_CORES
    NAP = ((per + NMM - 1) // NMM) * NMM

    key = (N, S, NB, st["T1"], round(c0, 9), round(as2, 12),
           round(ad2, 12), round(b2f, 9))
    if key not in _COMPILED:
        _COMPILED[key] = (_build_a(NAP), _build_b(S, NB, c0),
                          _build_c(S, NB, as2, ad2, b2f))
    nca, ncb, ncc = _COMPILED[key]

    trace = os.environ.get("GAT_TRACE", "0") == "1"
    if trace:
        _install_ntff_shim()

    # ---- launch A
    xbf = x.astype(ml_dtypes.bfloat16)
    in_a = []
    for c in range(N_CORES):
        lo = c * per
        xt = np.zeros((P, NAP), dtype=ml_dtypes.bfloat16)
        n_c = min(per, N - lo)
        xt[:, :n_c] = xbf[lo:lo + n_c].T
        in_a.append({"xt": xt, "wsc": np.asarray(wsc)})
    res_a = bass_utils.run_bass_kernel_spmd(
        nca, in_a, core_ids=list(range(N_CORES)), trace=trace)

    # host: assemble padded payload table [12, N+1] (col N = padding)
    pn_bf = np.zeros((12, N + 1), dtype=ml_dtypes.bfloat16)
    for c in range(N_CORES):
        lo = c * per
        n_c = min(per, N - lo)
        pn_bf[:, lo:lo + n_c] = res_a.results[c]["pn"][:, :n_c]
    pn_bf[0:4, N] = KILL

    # ---- launch B inputs: gather payloads into slot grids
    in_b = []
    for c in range(N_CORES):
        sub = pn_bf[:, st["srcgrids"][c]]            # [12, R, 128]
        edr = pn_bf[8:12][:, st["dstgrids"][c]]      # [4, R, 128]
        big = np.stack([sub[0:4], edr, sub[4:8]], axis=0)   # [sec, h, R, l]
        big = big.reshape(3, 4, S, P, P)
        gall = np.ascontiguousarray(
            big.transpose(3, 2, 0, 1, 4)).reshape(P, S * 3 * 4 * P)
        in_b.append({"gall": gall, "indt": np.asarray(st["indt"])})
    res_b = bass_utils.run_bass_kernel_spmd(
        ncb, in_b, core_ids=list(range(N_CORES)), trace=trace)

    # host: scatter h2 back to node order, with kill/zero padded tables
    h2_node = np.zeros(N, dtype=np.float32)
    for c in range(N_CORES):
        h2v = res_b.results[c]["h2"].reshape(-1)     # [NB*P] block-major
        real = st["perms"][c] >= 0
        h2_node[st["perms"][c][real]] = h2v[real]
    h2_kill = np.zeros(N + 1, dtype=np.float32)
    h2_kill[:N] = h2_node
    h2_kill[N] = 2.0 * KILL / as2
    h2_zero = np.zeros(N + 1, dtype=np.float32)
    h2_zero[:N] = h2_node
    h2k_bf = h2_kill.astype(ml_dtypes.bfloat16)
    h2z_bf = h2_zero.astype(ml_dtypes.bfloat16)

    # ---- launch C inputs
    in_c = []
    for c in range(N_CORES):
        g2 = h2k_bf[st["srcgrids"][c]].reshape(S, P, P)
        hr = h2z_bf[st["dstgrids"][c]].reshape(S, P, P)
        ga = np.stack([g2, hr], axis=0)              # [t, s, p, l]
        g2all = np.ascontiguousarray(
            ga.transpose(2, 1, 0, 3)).reshape(P, S * 2 * P)
        in_c.append({"g2all": g2all, "indt": np.asarray(st["indt"])})
    res_c = bass_utils.run_bass_kernel_spmd(
        ncc, in_c, core_ids=list(range(N_CORES)), trace=trace)

    out = np.zeros((N, 1), dtype=np.float32)
    for c in range(N_CORES):
        ov = res_c.results[c]["out"].reshape(-1)
        real = st["perms"][c] >= 0
        out[st["perms"][c][real], 0] = ov[real]

    ts = [r.exec_time_ns for r in (res_a, res_b, res_c)]
    LAST_EXEC_NS = sum(t for t in ts if t) if any(ts) else None
    LAST_RESULTS = (res_a, res_b, res_c)
    return out
